# revision 1
# baseline (speedup 1.0000x reference)
"""Trainium2 Bass kernel for nn_Net_48301202211072 (GNN message passing).

2-layer GraphConv + TopKPooling + readout + MLP head, sharded over 8
NeuronCores. Strategy:

- Nodes (and their incident edges, grouped by destination) are sharded
  across cores; x and the small weights are replicated.
- segment_sum: host packs each core's edges into dst-bins of 128 nodes,
  8-slot groups per dst (padded). Device gathers source rows with
  dma_gather, applies per-edge weights + segmented 8-sums on DVE, then a
  0/1 one-hot matmul on the TensorEngine reduces group partials into
  per-dst-bin aggregates (transposed: [feature, dst]).
- TopKPooling stays in original node indexing via masks: a 5-stage
  64-bin histogram over scores finds the exact k-th threshold on every
  core (replicated, deterministic); kept nodes are scaled by tanh score.
- Layer-2 gathers come from an AllGather'd table of scaled features.
- Readout (masked max + mean) via PE transposes + DVE reductions,
  combined across cores with a single AllGather per layer.
- The MLP head is sharded: lin1/lin2 by output rows, lin3 by
  contraction (each core's lin2 shard is exactly its lin3 slice), with
  one AllReduce at the end.

Everything is fp32.
"""
import dataclasses
import math
import sys

import numpy as np

sys.path.insert(0, "/opt/trn_rl_repo")

import concourse.bacc as bacc  # noqa: E402
import concourse.mybir as mybir  # noqa: E402
import concourse.tile as tile  # noqa: E402
from concourse import bass_utils  # noqa: E402

FP32 = mybir.dt.float32
BF16 = mybir.dt.bfloat16
I16 = mybir.dt.int16
AX = mybir.AxisListType
OP = mybir.AluOpType
ACT = mybir.ActivationFunctionType

NCORES = 8
N = 10000
FIN = 256
HID = 500
HPAD = 512
NOUT = 100
NPC = N // NCORES          # 1250 nodes per core
NCH = 10                   # dst bins per core (128 nodes each)
NPAD = NCH * 128           # 1280
NBINS = 64
NSTAGES = 5
K1 = N // 2
K2 = N // 4
L1O, L2O, L3I = 2000, 4000, 4000
L1S = L1O // NCORES        # 250 lin1 rows per core
L2S = L2O // NCORES        # 500 lin2 rows per core
L3S = L3I // NCORES        # 500 lin3 contraction cols per core
BIG = 1e30


# ---------------------------------------------------------------------------
# host preprocessing
# ---------------------------------------------------------------------------

def _pack(edge_src, edge_dst, edge_weight):
    src = np.asarray(edge_src, np.int64)
    dst = np.asarray(edge_dst, np.int64)
    w = np.asarray(edge_weight, np.float32)

    cores = []
    for c in range(NCORES):
        lo = c * NPC
        m = (dst >= lo) & (dst < lo + NPC)
        es, ed, ew = src[m], dst[m] - lo, w[m]
        order = np.argsort(ed, kind="stable")
        es, ed, ew = es[order], ed[order], ew[order]
        deg = np.bincount(ed, minlength=NPC)
        ngroups = (deg + 7) // 8
        starts = np.zeros(NPC + 1, np.int64)
        np.cumsum(deg, out=starts[1:])
        cores.append(dict(es=es, ew=ew, deg=deg, ng=ngroups, starts=starts))

    def try_pack(ci, caps):
        order = np.argsort(-ci["ng"], kind="stable")
        bins = [[] for _ in range(NCH)]
        bg = np.zeros(NCH, np.int64)
        for d in order:
            ok = -1
            for b in range(NCH):
                if len(bins[b]) < 128 and bg[b] + ci["ng"][d] <= caps[b] * 128:
                    ok = b
                    break
            if ok < 0:
                return None
            bins[ok].append(d)
            bg[ok] += ci["ng"][d]
        return bins

    cap_options = [[3] * 5 + [2] * 5, [3] * 6 + [2] * 4, [3] * 8 + [2] * 2,
                   [3] * 10, [4] * 10, [6] * 10, [10] * 10]
    caps, packs = None, None
    for co in cap_options:
        ps = []
        for c in range(NCORES):
            r = try_pack(cores[c], co)
            if r is None:
                ps = None
                break
            ps.append(r)
        if ps is not None:
            caps, packs = co, ps
            break
    assert caps is not None, "bin packing failed"
    BTOT = int(sum(caps))

    prep = []
    for c in range(NCORES):
        ci = cores[c]
        bins = packs[c]
        nodemap = np.full(NPAD, -1, np.int64)
        for b in range(NCH):
            for i, d in enumerate(bins[b]):
                nodemap[b * 128 + i] = d
        padmask = (nodemap >= 0).astype(np.float32)
        binpos = np.full(NPC, -1, np.int64)
        for i, d in enumerate(nodemap):
            if d >= 0:
                binpos[d] = i

        idx1 = np.zeros(BTOT * 1024, np.int64)
        wsl = np.zeros((128, BTOT * 8), np.float32)
        gdst = np.zeros((128, BTOT), np.float32)
        blk0 = 0
        for b in range(NCH):
            q = 0
            for i, d in enumerate(bins[b]):
                st, en = ci["starts"][d], ci["starts"][d + 1]
                for gi in range(int(ci["ng"][d])):
                    p, blk = q % 128, blk0 + q // 128
                    gdst[p, blk] = i
                    base = st + gi * 8
                    nreal = min(8, en - base)
                    for j in range(nreal):
                        # slot position: block blk, col j, partition p
                        idx1[blk * 1024 + j * 128 + p] = ci["es"][base + j]
                        wsl[p, blk * 8 + j] = ci["ew"][base + j]
                    q += 1
            blk0 += caps[b]
        prep.append(dict(nodemap=nodemap, padmask=padmask, binpos=binpos,
                         idx1=idx1, wsl=wsl, gdst=gdst))

    # L2 index remap: global node n -> row in AllGather'd g1 table
    binpos_all = np.stack([p["binpos"] for p in prep])  # [NCORES, NPC]
    for c in range(NCORES):
        i1 = prep[c]["idx1"]
        cc = i1 // NPC
        prep[c]["idx2"] = cc * NPAD + binpos_all[cc, i1 - cc * NPC]

    return dict(caps=list(caps), BTOT=BTOT), prep


def _wrap16(idx_flat, BTOT):
    """[BTOT*1024] -> [128, BTOT*64] int16, per-block wrapped-16 replicated."""
    out = np.zeros((128, BTOT * 64), np.int16)
    for blk in range(BTOT):
        b = idx_flat[blk * 1024:(blk + 1) * 1024].astype(np.int16)
        t = b.reshape(64, 16).T          # [16, 64]
        out[:, blk * 64:(blk + 1) * 64] = np.tile(t, (8, 1))
    return out


def _host_inputs(inputs, cfg, prep):
    BTOT = cfg["BTOT"]
    x = np.ascontiguousarray(np.asarray(inputs["x"], np.float32))

    def padT(a, rows, cols):
        out = np.zeros((rows, cols), np.float32)
        t = np.asarray(a, np.float32).T
        out[: t.shape[0], : t.shape[1]] = t
        return out

    w1relT = padT(inputs["W1_rel"], FIN, HPAD)
    w1rootT = padT(inputs["W1_root"], FIN, HPAD)
    w2relT = padT(inputs["W2_rel"], HPAD, HPAD)
    w2rootT = padT(inputs["W2_root"], HPAD, HPAD)

    def repl(v, cols):
        out = np.zeros((128, cols), np.float32)
        vv = np.asarray(v, np.float32)
        out[:, : vv.shape[0]] = vv[None, :]
        return out

    b1r = repl(inputs["b1"], HPAD)
    b2r = repl(inputs["b2"], HPAD)
    p1r = repl(inputs["p1_w"], HPAD)
    p2r = repl(inputs["p2_w"], HPAD)

    iota128 = np.tile(np.arange(128, dtype=np.float32)[None, :], (128, 1))
    iotaB = np.tile(np.arange(NBINS, dtype=np.float32)[None, :], (128, 1))
    ident = np.eye(128, dtype=np.float32)
    ones1x128 = np.ones((1, 128), np.float32)
    onesP = np.ones((128, 1), np.float32)

    lin1W = np.asarray(inputs["lin1_W"], np.float32)   # [2000, 1000]
    lin2W = np.asarray(inputs["lin2_W"], np.float32)   # [4000, 2000]
    lin3W = np.asarray(inputs["lin3_W"], np.float32)   # [100, 4000]
    lin1b = np.asarray(inputs["lin1_b"], np.float32)
    lin2b = np.asarray(inputs["lin2_b"], np.float32)
    lin3b = np.asarray(inputs["lin3_b"], np.float32)

    per_core = []
    for c in range(NCORES):
        pr = prep[c]
        # xT in bin-permuted order
        xT = np.zeros((FIN, NPAD), np.float32)
        nm = pr["nodemap"]
        real = nm >= 0
        xT[:, real] = x[c * NPC + nm[real]].T

        # head shards (padded layouts; see kernel program for index math)
        l1T = np.zeros((1024, 256), np.float32)
        sh = lin1W[c * L1S:(c + 1) * L1S].T            # [1000, 250]
        l1T[:500, :250] = sh[:500]
        l1T[512:1012, :250] = sh[500:]
        b1h = np.zeros((128, 2), np.float32)
        b1h.T.flat[:L1S] = lin1b[c * L1S:(c + 1) * L1S]

        l2T = np.zeros((2048, 500), np.float32)
        sh2 = lin2W[c * L2S:(c + 1) * L2S].T           # [2000, 500]
        for cc in range(NCORES):
            l2T[cc * 256: cc * 256 + 250] = sh2[cc * 250:(cc + 1) * 250]
        b2h = np.zeros((128, 4), np.float32)
        b2h.T.flat[:L2S] = lin2b[c * L2S:(c + 1) * L2S]

        l3T = np.zeros((512, 128), np.float32)
        l3T[:500, :NOUT] = lin3W[:, c * L3S:(c + 1) * L3S].T
        b3h = np.zeros((128, 1), np.float32)
        b3h[:NOUT, 0] = lin3b

        per_core.append(dict(
            xtbl=x,
            idx1=_wrap16(pr["idx1"], BTOT),
            idx2=_wrap16(pr["idx2"], BTOT),
            wsl=pr["wsl"],
            gdst=pr["gdst"],
            padmask=np.ascontiguousarray(
                pr["padmask"].reshape(NCH, 128).T.astype(np.float32)),
            xT=xT,
            w1relT=w1relT, w1rootT=w1rootT, w2relT=w2relT, w2rootT=w2rootT,
            b1r=b1r, b2r=b2r, p1r=p1r, p2r=p2r,
            iota128=iota128, iotaB=iotaB, ident=ident,
            ones1x128=ones1x128, onesP=onesP,
            l1T=l1T, b1h=b1h, l2T=l2T, b2h=b2h, l3T=l3T, b3h=b3h,
        ))
    return per_core


# ---------------------------------------------------------------------------
# device program
# ---------------------------------------------------------------------------

def _mid_bcast(ap, n, axis=1):
    """insert a step-0 dim of size n at position `axis` (free dims only)."""
    ap = ap.unsqueeze(axis)
    newap = list(ap.ap)
    newap[axis] = [0, n]
    return dataclasses.replace(ap, ap=newap)


def _build(cfg):
    caps, BTOT = cfg["caps"], cfg["BTOT"]
    nc = bacc.Bacc("TRN2", target_bir_lowering=False, debug=False,
                   num_devices=NCORES)

    def din(name, shape, dt=FP32):
        return nc.dram_tensor(name, shape, dt, kind="ExternalInput")

    xtbl = din("xtbl", [N, FIN])
    idx1 = din("idx1", [128, BTOT * 64], I16)
    idx2 = din("idx2", [128, BTOT * 64], I16)
    wsl = din("wsl", [128, BTOT * 8])
    gdst = din("gdst", [128, BTOT])
    padmask = din("padmask", [128, NCH])
    xT = din("xT", [FIN, NPAD])
    w1relT = din("w1relT", [FIN, HPAD])
    w1rootT = din("w1rootT", [FIN, HPAD])
    w2relT = din("w2relT", [HPAD, HPAD])
    w2rootT = din("w2rootT", [HPAD, HPAD])
    b1r = din("b1r", [128, HPAD])
    b2r = din("b2r", [128, HPAD])
    p1r = din("p1r", [128, HPAD])
    p2r = din("p2r", [128, HPAD])
    iota128 = din("iota128", [128, 128])
    iotaB = din("iotaB", [128, NBINS])
    ident = din("ident", [128, 128])
    ones1x128 = din("ones1x128", [1, 128])
    onesP = din("onesP", [128, 1])
    l1T = din("l1T", [1024, 256])
    b1h = din("b1h", [128, 2])
    l2T = din("l2T", [2048, 500])
    b2h = din("b2h", [128, 4])
    l3T = din("l3T", [512, 128])
    b3h = din("b3h", [128, 1])

    out = nc.dram_tensor("out", [1, NOUT], FP32, kind="ExternalOutput")

    RG = [list(range(NCORES))]

    with tile.TileContext(nc) as tc:
        with (
            tc.tile_pool(name="const", bufs=1) as cp,
            tc.tile_pool(name="gather", bufs=2) as gp,
            tc.tile_pool(name="work", bufs=1) as wp,
            tc.tile_pool(name="big", bufs=1) as bigp,
            tc.tile_pool(name="psA", bufs=2, space="PSUM") as psA,
            tc.tile_pool(name="psB", bufs=2, space="PSUM") as psB,
            tc.tile_pool(name="psS", bufs=1, space="PSUM") as psS,
            tc.tile_pool(name="dram", bufs=1, space="DRAM") as dr,
        ):
            def load(t, src, dt=FP32, tag=None):
                tl = cp.tile(list(src.shape), dt, tag=tag or src.name)
                nc.sync.dma_start(tl[:], src[:])
                return tl

            idx1_t = load(idx1, idx1, I16)
            idx2_t = load(idx2, idx2, I16)
            wsl_t = load(wsl, wsl)
            gdst_t = load(gdst, gdst)
            pad_t = load(padmask, padmask)
            io_t = load(iota128, iota128)
            iob_t = load(iotaB, iotaB)
            id_t = load(ident, ident)
            ones_t = load(ones1x128, ones1x128)
            onesP_t = load(onesP, onesP)
            b1_t = load(b1r, b1r)
            b2_t = load(b2r, b2r)
            p1_t = load(p1r, p1r)
            p2_t = load(p2r, p2r)

            def load_chunks(src, nchunks, cols, tag):
                ts = []
                for k in range(nchunks):
                    t = cp.tile([128, cols], FP32, tag=f"{tag}{k}")
                    nc.sync.dma_start(t[:], src[k * 128:(k + 1) * 128, :cols])
                    ts.append(t)
                return ts

            w1rel_t = load_chunks(w1relT, 2, HPAD, "w1rel")
            w1root_t = load_chunks(w1rootT, 2, HPAD, "w1root")
            w2rel_t = load_chunks(w2relT, 4, HPAD, "w2rel")
            w2root_t = load_chunks(w2rootT, 4, HPAD, "w2root")
            xT_t = load_chunks(xT, 2, NPAD, "xTc")

            # DRAM internal tiles
            zsh1 = dr.tile([NPAD, 1], FP32)
            zag1 = dr.tile([NCORES * NPAD, 1], FP32, addr_space="Shared")
            zsh2 = dr.tile([NPAD, 1], FP32)
            zag2 = dr.tile([NCORES * NPAD, 1], FP32, addr_space="Shared")
            g1sh = dr.tile([NPAD, HPAD], FP32)
            g1ag = dr.tile([NCORES * NPAD, HPAD], FP32, addr_space="Shared")
            ro1in = dr.tile([2, HPAD], FP32)
            ro1ag = dr.tile([2 * NCORES, HPAD], FP32, addr_space="Shared")
            ro2in = dr.tile([2, HPAD], FP32)
            ro2ag = dr.tile([2 * NCORES, HPAD], FP32, addr_space="Shared")
            z1hsh = dr.tile([256, 1], FP32)
            z1hag = dr.tile([256 * NCORES, 1], FP32, addr_space="Shared")
            oin = dr.tile([128, 1], FP32)
            oar = dr.tile([128, 1], FP32, addr_space="Shared")

            # ---------------- conv layer (aggregation + dense) -------------
            def conv_layer(F, tbl_ap, idx_t, wrel_t, wroot_t, rootT_t, b_t,
                           p_t, h_all, z_all, lname):
                """aggregation via gather + segmented sum + one-hot matmul,
                then dense; fills h_all [128, NCH*HPAD] and z_all [128, NCH].
                rootT_t: list of [128, NPAD] lhsT tiles for the root term."""
                nfc = F // 128
                aggT = [bigp.tile([128, NPAD], FP32, tag=f"aggT{fc}",
                                  name=f"aggT{lname}{fc}")
                        for fc in range(nfc)]
                blk0 = 0
                for b in range(NCH):
                    agg_ps = psA.tile([128, 512], FP32, tag="aggps")
                    nblk = caps[b]
                    for k in range(nblk):
                        blk = blk0 + k
                        gt = gp.tile([128, 8, F], FP32, tag="gath")
                        nc.gpsimd.dma_gather(
                            gt[:], tbl_ap, idx_t[:, blk * 64:(blk + 1) * 64],
                            1024, 1024, F)
                        wb = wsl_t[:, blk * 8:(blk + 1) * 8] \
                            .unsqueeze(2).broadcast_to([128, 8, F])
                        nc.vector.tensor_tensor(out=gt[:], in0=gt[:], in1=wb,
                                                op=OP.mult)
                        pt = wp.tile([128, F], FP32, tag="part", bufs=3)
                        nc.vector.tensor_reduce(
                            out=pt[:], in_=gt[:].rearrange("p c f -> p f c"),
                            op=OP.add, axis=AX.X)
                        oh = wp.tile([128, 128], FP32, tag="oh", bufs=3)
                        nc.vector.tensor_tensor(
                            out=oh[:],
                            in0=gdst_t[:, blk:blk + 1].broadcast_to([128, 128]),
                            in1=io_t[:], op=OP.is_equal)
                        for fc in range(nfc):
                            nc.tensor.matmul(
                                out=agg_ps[:, fc * 128:(fc + 1) * 128],
                                lhsT=pt[:, fc * 128:(fc + 1) * 128],
                                rhs=oh[:], start=(k == 0 and fc == 0),
                                stop=(k == nblk - 1 and fc == nfc - 1))
                    for fc in range(nfc):
                        nc.vector.tensor_copy(
                            aggT[fc][:, b * 128:(b + 1) * 128],
                            agg_ps[:, fc * 128:(fc + 1) * 128])
                    blk0 += nblk

                # dense: h = relu(aggT.T @ wrelT + root.T @ wrootT + b)
                for b in range(NCH):
                    hp = psB.tile([128, HPAD], FP32, tag="hps")
                    for fc in range(nfc):
                        nc.tensor.matmul(
                            out=hp[:], lhsT=aggT[fc][:, b * 128:(b + 1) * 128],
                            rhs=wrel_t[fc][:], start=(fc == 0), stop=False)
                    nroot = len(rootT_t)
                    for fc in range(nroot):
                        nc.tensor.matmul(
                            out=hp[:], lhsT=rootT_t[fc][:, b * 128:(b + 1) * 128],
                            rhs=wroot_t[fc][:], start=False, stop=(fc == nroot - 1))
                    hc = h_all[:, b * HPAD:(b + 1) * HPAD]
                    nc.vector.tensor_tensor(out=hc, in0=hp[:], in1=b_t[:],
                                            op=OP.add)
                    nc.vector.tensor_scalar_max(hc, hc, 0.0)
                    # score z = h . p
                    scr = wp.tile([128, HPAD], FP32, tag="scr", bufs=2)
                    nc.vector.tensor_tensor(out=scr[:], in0=hc, in1=p_t[:],
                                            op=OP.mult)
                    nc.vector.tensor_reduce(out=z_all[:, b:b + 1], in_=scr[:],
                                            op=OP.add, axis=AX.X)

            # ---------------- histogram k-th threshold ---------------------
            def topk_tau(zag, k, lname):
                """returns [128,1] tile with the k-th-largest threshold."""
                zt = wp.tile([128, NCORES * NPAD // 128], FP32, tag="zt")
                nfree = NCORES * NPAD // 128
                nc.sync.dma_start(
                    zt[:], zag[:].rearrange("(p f) o -> p (f o)", p=128))
                # min over real entries (pads are -1e30), max overall
                mm = wp.tile([128, 2], FP32, tag="mm")
                msk = wp.tile([128, nfree], FP32, tag="hmsk")
                nc.vector.tensor_scalar(msk[:], zt[:], -1e29, 2e30, OP.is_lt,
                                        OP.mult)
                nc.vector.tensor_tensor(out=msk[:], in0=msk[:], in1=zt[:],
                                        op=OP.add)
                nc.vector.tensor_reduce(out=mm[:, 0:1], in_=msk[:], op=OP.min,
                                        axis=AX.X)
                nc.vector.tensor_reduce(out=mm[:, 1:2], in_=zt[:], op=OP.max,
                                        axis=AX.X)
                lw = wp.tile([1, 2], FP32, tag="lw")  # [lo, w]
                mmT = wp.tile([1, 2, 128], FP32, tag="mmTs")
                for col in range(2):
                    mmT_ps = psS.tile([1, 128], FP32, tag="small")
                    nc.tensor.transpose(out=mmT_ps[:], in_=mm[:, col:col + 1],
                                        identity=id_t[:])
                    nc.vector.tensor_copy(mmT[:, col, :], mmT_ps[:])
                nc.vector.tensor_reduce(out=lw[:, 0:1], in_=mmT[:, 0, :],
                                        op=OP.min, axis=AX.X)
                nc.vector.tensor_reduce(out=lw[:, 1:2], in_=mmT[:, 1, :],
                                        op=OP.max, axis=AX.X)
                # lo -= 1e-3 ; w = (hi - lo) * (1/64)
                nc.vector.tensor_scalar_add(lw[:, 0:1], lw[:, 0:1], -1e-3)
                nc.vector.tensor_scalar_add(lw[:, 1:2], lw[:, 1:2], 1e-3)
                nc.vector.tensor_tensor(out=lw[:, 1:2], in0=lw[:, 1:2],
                                        in1=lw[:, 0:1], op=OP.subtract)
                nc.vector.tensor_scalar_mul(lw[:, 1:2], lw[:, 1:2], 1.0 / NBINS)

                for st in range(NSTAGES):
                    lwb_ps = psS.tile([128, 2], FP32, tag="small")
                    nc.tensor.matmul(out=lwb_ps[:], lhsT=ones_t[:], rhs=lw[:],
                                     start=True, stop=True)
                    lwb = wp.tile([128, 2], FP32, tag="lwbs")
                    nc.vector.tensor_copy(lwb[:], lwb_ps[:])
                    tt = wp.tile([128, NBINS], FP32, tag="tt")
                    nc.vector.tensor_scalar(tt[:], iob_t[:], lwb[:, 1:2],
                                            lwb[:, 0:1], OP.mult, OP.add)
                    S = wp.tile([128, nfree, NBINS], BF16, tag="S")
                    nc.vector.tensor_tensor(
                        out=S[:],
                        in0=zt[:].unsqueeze(2).broadcast_to([128, nfree, NBINS]),
                        in1=_mid_bcast(tt[:], nfree), op=OP.is_ge)
                    cntp = wp.tile([128, NBINS], FP32, tag="cntp")
                    nc.vector.tensor_reduce(
                        out=cntp[:], in_=S[:].rearrange("p n j -> p j n"),
                        op=OP.add, axis=AX.X)
                    cnt_ps = psS.tile([1, NBINS], FP32, tag="small")
                    nc.tensor.matmul(out=cnt_ps[:], lhsT=onesP_t[:],
                                     rhs=cntp[:], start=True, stop=True)
                    fl = wp.tile([1, NBINS], FP32, tag="fl")
                    nc.vector.tensor_scalar(fl[:], cnt_ps[:], float(k), None,
                                            OP.is_ge)
                    js = wp.tile([1, 1], FP32, tag="js")
                    nc.vector.tensor_reduce(out=js[:], in_=fl[:], op=OP.add,
                                            axis=AX.X)
                    nc.vector.tensor_scalar_add(js[:], js[:], -1.0)
                    # lo = js * w + lo ; w = w / 64
                    nc.vector.tensor_scalar(lw[:, 0:1], js[:], lw[:, 1:2],
                                            lw[:, 0:1], OP.mult, OP.add)
                    if st != NSTAGES - 1:
                        nc.vector.tensor_scalar_mul(lw[:, 1:2], lw[:, 1:2],
                                                    1.0 / NBINS)
                taub_ps = psS.tile([128, 1], FP32, tag="small")
                nc.tensor.matmul(out=taub_ps[:], lhsT=ones_t[:], rhs=lw[:, 0:1],
                                 start=True, stop=True)
                taub = wp.tile([128, 1], FP32, tag=f"taubs{lname}")
                nc.vector.tensor_copy(taub[:], taub_ps[:])
                return taub

            def inv_norm_b(p_t, lname):
                """[128,1] broadcast of 1/||p||."""
                sq = wp.tile([1, HPAD], FP32, tag="pnsq")
                nc.vector.tensor_tensor(out=sq[:], in0=p_t[0:1, :],
                                        in1=p_t[0:1, :], op=OP.mult)
                n2 = wp.tile([1, 1], FP32, tag="pn2")
                nc.vector.tensor_reduce(out=n2[:], in_=sq[:], op=OP.add,
                                        axis=AX.X)
                nc.scalar.activation(n2[:], n2[:], ACT.Sqrt)
                nc.vector.reciprocal(n2[:], n2[:])
                ib_ps = psS.tile([128, 1], FP32, tag="small")
                nc.tensor.matmul(out=ib_ps[:], lhsT=ones_t[:], rhs=n2[:],
                                 start=True, stop=True)
                ib = wp.tile([128, 1], FP32, tag=f"invbs{lname}")
                nc.vector.tensor_copy(ib[:], ib_ps[:])
                return ib

            # ======================= layer 1 ===============================
            h1 = bigp.tile([128, NCH * HPAD], FP32, tag="h_all")
            z1 = wp.tile([128, NCH], FP32, tag="z1")
            conv_layer(FIN, xtbl[:], idx1_t, w1rel_t, w1root_t, xT_t,
                       b1_t, p1_t, h1[:], z1[:], "l1")

            inv1b = inv_norm_b(p1_t, "l1")
            s1 = wp.tile([128, NCH], FP32, tag="s1")
            nc.scalar.activation(s1[:], z1[:], ACT.Tanh, scale=inv1b[:, 0:1])

            # masked z for selection/padding
            pm30 = wp.tile([128, NCH], FP32, tag="pm30")
            nc.vector.tensor_scalar(pm30[:], pad_t[:], 1.0, BIG, OP.subtract,
                                    OP.mult)
            zm1 = wp.tile([128, NCH], FP32, tag="zm1")
            nc.vector.tensor_tensor(out=zm1[:], in0=z1[:], in1=pad_t[:],
                                    op=OP.mult)
            nc.vector.tensor_tensor(out=zm1[:], in0=zm1[:], in1=pm30[:],
                                    op=OP.add)
            nc.sync.dma_start(
                zsh1[:].rearrange("(b p) o -> p (b o)", p=128), zm1[:])
            nc.gpsimd.collective_compute(
                "AllGather", OP.bypass, replica_groups=RG,
                ins=[zsh1[:]], outs=[zag1[:]])

            tau1b = topk_tau(zag1, K1, "l1")
            kp1 = wp.tile([128, NCH], FP32, tag="kp1")
            nc.vector.tensor_scalar(kp1[:], zm1[:], tau1b[:, 0:1], None,
                                    OP.is_ge)
            a1 = wp.tile([128, NCH], FP32, tag="a1")
            nc.vector.tensor_tensor(out=a1[:], in0=s1[:], in1=kp1[:],
                                    op=OP.mult)
            km30 = wp.tile([128, NCH], FP32, tag="km30")
            nc.vector.tensor_scalar(km30[:], kp1[:], 1.0, BIG, OP.subtract,
                                    OP.mult)

            # g1 (+ masked transpose) + readout 1
            gmT1 = [bigp.tile([128, NPAD], FP32, tag=f"gmT{fc}",
                             name=f"gmT1_{fc}")
                    for fc in range(4)]
            ro1s_ps = psS.tile([1, HPAD], FP32, tag="rosum")
            for b in range(NCH):
                hc = h1[:, b * HPAD:(b + 1) * HPAD]
                g1c = wp.tile([128, HPAD], FP32, tag="g1c", bufs=2)
                nc.vector.tensor_scalar(g1c[:], hc, a1[:, b:b + 1], None,
                                        OP.mult)
                nc.sync.dma_start(g1sh[b * 128:(b + 1) * 128, :], g1c[:])
                nc.tensor.matmul(out=ro1s_ps[:], lhsT=onesP_t[:], rhs=g1c[:],
                                 start=(b == 0), stop=(b == NCH - 1))
                gmc = wp.tile([128, HPAD], FP32, tag="gmc", bufs=2)
                nc.vector.tensor_scalar(gmc[:], hc, a1[:, b:b + 1],
                                        km30[:, b:b + 1], OP.mult, OP.add)
                for fc in range(4):
                    tp = psB.tile([128, 128], FP32, tag="trp")
                    nc.tensor.transpose(out=tp[:],
                                        in_=gmc[:, fc * 128:(fc + 1) * 128],
                                        identity=id_t[:])
                    nc.vector.tensor_copy(gmT1[fc][:, b * 128:(b + 1) * 128],
                                          tp[:])
            nc.gpsimd.collective_compute(
                "AllGather", OP.bypass, replica_groups=RG,
                ins=[g1sh[:]], outs=[g1ag[:]])

            m1T = wp.tile([128, 4], FP32, tag="m1T")
            for fc in range(4):
                nc.vector.tensor_reduce(out=m1T[:, fc:fc + 1], in_=gmT1[fc][:],
                                        op=OP.max, axis=AX.X)
            ro1s = wp.tile([1, HPAD], FP32, tag="ro1s")
            nc.vector.tensor_copy(ro1s[:], ro1s_ps[:])
            nc.sync.dma_start(ro1in[0:1, :], ro1s[:])
            nc.sync.dma_start(
                ro1in[1:2, :].rearrange("o (c p) -> p (o c)", p=128), m1T[:])
            nc.gpsimd.collective_compute(
                "AllGather", OP.bypass, replica_groups=RG,
                ins=[ro1in[:]], outs=[ro1ag[:]])

            # ======================= layer 2 ===============================
            h2 = bigp.tile([128, NCH * HPAD], FP32, tag="h_all")
            z2 = wp.tile([128, NCH], FP32, tag="z2")
            conv_layer(HPAD, g1ag[:], idx2_t, w2rel_t, w2root_t, gmT1,
                       b2_t, p2_t, h2[:], z2[:], "l2")

            inv2b = inv_norm_b(p2_t, "l2")
            s2 = wp.tile([128, NCH], FP32, tag="s2")
            nc.scalar.activation(s2[:], z2[:], ACT.Tanh, scale=inv2b[:, 0:1])
            zm2 = wp.tile([128, NCH], FP32, tag="zm2")
            nc.vector.tensor_tensor(out=zm2[:], in0=z2[:], in1=kp1[:],
                                    op=OP.mult)
            nc.vector.tensor_tensor(out=zm2[:], in0=zm2[:], in1=km30[:],
                                    op=OP.add)
            nc.sync.dma_start(
                zsh2[:].rearrange("(b p) o -> p (b o)", p=128), zm2[:])
            nc.gpsimd.collective_compute(
                "AllGather", OP.bypass, replica_groups=RG,
                ins=[zsh2[:]], outs=[zag2[:]])

            tau2b = topk_tau(zag2, K2, "l2")
            kp2 = wp.tile([128, NCH], FP32, tag="kp2")
            nc.vector.tensor_scalar(kp2[:], zm2[:], tau2b[:, 0:1], None,
                                    OP.is_ge)
            a2 = wp.tile([128, NCH], FP32, tag="a2")
            nc.vector.tensor_tensor(out=a2[:], in0=s2[:], in1=kp2[:],
                                    op=OP.mult)
            km30b = wp.tile([128, NCH], FP32, tag="km30b")
            nc.vector.tensor_scalar(km30b[:], kp2[:], 1.0, BIG, OP.subtract,
                                    OP.mult)

            ro2s_ps = psS.tile([1, HPAD], FP32, tag="rosum")
            m2T = wp.tile([128, 4], FP32, tag="m2T")
            nc.vector.memset(m2T[:], -1e30)
            for b in range(NCH):
                hc = h2[:, b * HPAD:(b + 1) * HPAD]
                g2c = wp.tile([128, HPAD], FP32, tag="g1c", bufs=2)
                nc.vector.tensor_scalar(g2c[:], hc, a2[:, b:b + 1], None,
                                        OP.mult)
                nc.tensor.matmul(out=ro2s_ps[:], lhsT=onesP_t[:], rhs=g2c[:],
                                 start=(b == 0), stop=(b == NCH - 1))
                gmc = wp.tile([128, HPAD], FP32, tag="gmc", bufs=2)
                nc.vector.tensor_scalar(gmc[:], hc, a2[:, b:b + 1],
                                        km30b[:, b:b + 1], OP.mult, OP.add)
                for fc in range(4):
                    tp = psB.tile([128, 128], FP32, tag="trp")
                    nc.tensor.transpose(out=tp[:],
                                        in_=gmc[:, fc * 128:(fc + 1) * 128],
                                        identity=id_t[:])
                    red = wp.tile([128, 1], FP32, tag="redm", bufs=2)
                    nc.vector.tensor_reduce(out=red[:], in_=tp[:], op=OP.max,
                                            axis=AX.X)
                    nc.vector.tensor_tensor(out=m2T[:, fc:fc + 1],
                                            in0=m2T[:, fc:fc + 1], in1=red[:],
                                            op=OP.max)
            ro2s = wp.tile([1, HPAD], FP32, tag="ro2s")
            nc.vector.tensor_copy(ro2s[:], ro2s_ps[:])
            nc.sync.dma_start(ro2in[0:1, :], ro2s[:])
            nc.sync.dma_start(
                ro2in[1:2, :].rearrange("o (c p) -> p (o c)", p=128), m2T[:])
            nc.gpsimd.collective_compute(
                "AllGather", OP.bypass, replica_groups=RG,
                ins=[ro2in[:]], outs=[ro2ag[:]])

            # ======================= readout combine + head ================
            def combine(roag, kdiv, mxout, mnout):
                """[16, HPAD] AG -> maxT [128,4], meanT [128,4] (transposed)."""
                sums = wp.tile([128, 4, 2 * NCORES], FP32, tag="cmb")
                for r in range(2 * NCORES):
                    nc.sync.dma_start(
                        sums[:, :, r],
                        roag[r:r + 1, :].rearrange("o (c p) -> p (o c)", p=128))
                # layout: [p, c, r] where r covers 16 rows (8 shards x 2)
                s_ap = sums[:].rearrange("p c (s t) -> p c t s", t=2)
                nc.vector.tensor_reduce(out=mnout[:], in_=s_ap[:, :, 0, :],
                                        op=OP.add, axis=AX.X)
                nc.vector.tensor_reduce(out=mxout[:], in_=s_ap[:, :, 1, :],
                                        op=OP.max, axis=AX.X)
                nc.vector.tensor_scalar_mul(mnout[:], mnout[:], 1.0 / kdiv)

            mx1 = wp.tile([128, 4], FP32, tag="mx1")
            mn1 = wp.tile([128, 4], FP32, tag="mn1")
            combine(ro1ag, K1, mx1, mn1)
            mx2 = wp.tile([128, 4], FP32, tag="mx2")
            mn2 = wp.tile([128, 4], FP32, tag="mn2")
            combine(ro2ag, K2, mx2, mn2)

            zT = wp.tile([128, 8], FP32, tag="zT")
            nc.vector.tensor_tensor(out=zT[:, 0:4], in0=mx1[:], in1=mx2[:],
                                    op=OP.add)
            nc.vector.tensor_tensor(out=zT[:, 4:8], in0=mn1[:], in1=mn2[:],
                                    op=OP.add)

            # lin1: out1T [250(pad 256), 1] = l1T.T @ zT
            l1_t = load_chunks(l1T, 8, 256, "l1T")
            b1h_t = load(b1h, b1h)
            z1hT = wp.tile([128, 2], FP32, tag="z1hT")
            for m in range(2):
                o1p = psB.tile([128, 1], FP32, tag="hps")
                for t in range(8):
                    nc.tensor.matmul(out=o1p[:],
                                     lhsT=l1_t[t][:, m * 128:(m + 1) * 128],
                                     rhs=zT[:, t:t + 1],
                                     start=(t == 0), stop=(t == 7))
                nc.scalar.activation(z1hT[:, m:m + 1], o1p[:], ACT.Relu,
                                     bias=b1h_t[:, m:m + 1])
            nc.sync.dma_start(
                z1hsh[:].rearrange("(m p) o -> p (m o)", p=128), z1hT[:])
            nc.gpsimd.collective_compute(
                "AllGather", OP.bypass, replica_groups=RG,
                ins=[z1hsh[:]], outs=[z1hag[:]])

            # lin2: z2hT [500(pad 4x128), 1] = l2T.T @ z1full
            zh1 = wp.tile([128, 16], FP32, tag="zh1")
            nc.sync.dma_start(
                zh1[:], z1hag[:].rearrange("(c p) o -> p (c o)", p=128))
            l2_t = []
            for t in range(16):
                tl = cp.tile([128, 500], FP32, tag=f"l2T{t % 4}")
                nc.sync.dma_start(tl[:], l2T[t * 128:(t + 1) * 128, :])
                l2_t.append(tl)
            b2h_t = load(b2h, b2h)
            z2hT = wp.tile([128, 4], FP32, tag="z2hT")
            nc.vector.memset(z2hT[:], 0.0)
            msl = [(0, 128), (128, 256), (256, 384), (384, 500)]
            o2p = psB.tile([128, 4], FP32, tag="hps")
            for t in range(16):
                for m, (m0, m1) in enumerate(msl):
                    nc.tensor.matmul(out=o2p[: m1 - m0, m:m + 1],
                                     lhsT=l2_t[t][:, m0:m1],
                                     rhs=zh1[:, t:t + 1],
                                     start=(t == 0 and m == 0),
                                     stop=(t == 15 and m == 3))
            for m, (m0, m1) in enumerate(msl):
                nc.scalar.activation(z2hT[: m1 - m0, m:m + 1], o2p[: m1 - m0, m:m + 1],
                                     ACT.Relu, bias=b2h_t[: m1 - m0, m:m + 1])

            # lin3 partial: [100,1] += l3T.T @ z2hT  (own contraction shard)
            l3_t = load_chunks(l3T, 4, 128, "l3T")
            b3h_t = load(b3h, b3h)
            o3p = psB.tile([128, 1], FP32, tag="hps")
            for t in range(4):
                nc.tensor.matmul(out=o3p[:], lhsT=l3_t[t][:], rhs=z2hT[:, t:t + 1],
                                 start=(t == 0), stop=(t == 3))
            o3 = wp.tile([128, 1], FP32, tag="o3")
            nc.vector.tensor_copy(o3[:], o3p[:])
            nc.sync.dma_start(oin[:], o3[:])
            nc.gpsimd.collective_compute(
                "AllReduce", OP.add, replica_groups=RG,
                ins=[oin[:]], outs=[oar[:]])
            fin = wp.tile([128, 1], FP32, tag="fin")
            nc.sync.dma_start(fin[:], oar[:])
            nc.scalar.activation(fin[:], fin[:], ACT.Sigmoid,
                                 bias=b3h_t[:, 0:1])
            nc.sync.dma_start(out[:].rearrange("o f -> f o"), fin[:NOUT, :])

    nc.compile()
    return nc


# ---------------------------------------------------------------------------
# entry point
# ---------------------------------------------------------------------------

_CACHE = {}
TRACE = False


def kernel(**inputs):
    cfg, prep = _pack(inputs["edge_src"], inputs["edge_dst"],
                      inputs["edge_weight"])
    key = tuple(cfg["caps"])
    if key not in _CACHE:
        _CACHE[key] = _build(cfg)
    nc = _CACHE[key]
    in_maps = _host_inputs(inputs, cfg, prep)
    res = bass_utils.run_bass_kernel_spmd(
        nc, in_maps, core_ids=list(range(NCORES)), trace=TRACE)
    kernel.last_results = res
    return res.results[0]["out"]


if __name__ == "__main__":
    dat = np.load("/tmp/inputs.npz")
    inputs = {k: dat[k] for k in dat.files}
    got = kernel(**inputs)
    exp = np.load("/tmp/expected.npy")
    err = np.abs(got - exp).max()
    rel = err / np.abs(exp).max()
    print("out[0,:6] =", got[0, :6])
    print("exp[0,:6] =", exp[0, :6])
    print("max abs err:", err, "rel:", rel)



# revision 17
# speedup vs baseline: 1.6983x; 1.6983x over previous
"""Trainium2 Bass kernel for nn_Net_48301202211072 (GNN message passing).

2-layer GraphConv + TopKPooling + readout + MLP head, sharded over 8
NeuronCores. Strategy (v2):

- Nodes (and incident edges, grouped by destination) are sharded across
  cores. Edges are packed column-major into 11 destination bins per core
  (<=128 nodes and <=2048 edges per bin), 2 blocks of 1024 edge slots
  per bin. Everything is bf16 except score/threshold arithmetic.
- Layer-1 aggregation: the host pre-arranges per-edge source rows (xe)
  and edge-weighted one-hot matrices (Woh); the device streams both and
  aggregates with plain PE matmuls (no dma_gather, no per-edge DVE).
- Layer-2 aggregation: each node's table row [h1 | z1] is AllGather'd
  right after conv1 (before topk), so the big collective and the tau1
  histogram overlap; dma_gather fetches rows per edge and the per-slot
  scale tanh(z/||p||)*(z>=tau)*w is applied via the one-hot weights.
- TopK threshold: replicated 4-stage 64-bin histogram over the
  AllGather'd scores (bf16 compares, exact-enough within tolerance).
- Readout: ones-matmul mean + masked-transpose max, combined via small
  AllGathers (ro1 rides in the z2 AllGather payload).
- Head: lin1 replicated, lin2 sharded by rows, lin3 by contraction,
  one final AllReduce.
"""
import math
import sys

import numpy as np
import ml_dtypes

sys.path.insert(0, "/opt/trn_rl_repo")

import concourse.bacc as bacc  # noqa: E402
import concourse.mybir as mybir  # noqa: E402
import concourse.tile as tile  # noqa: E402
from concourse import bass_utils  # noqa: E402

FP32 = mybir.dt.float32
BF16 = mybir.dt.bfloat16
I16 = mybir.dt.int16
AX = mybir.AxisListType
OP = mybir.AluOpType
ACT = mybir.ActivationFunctionType
BFNP = ml_dtypes.bfloat16

NCORES = 8
N = 10000
FIN = 256
HID = 500
HPAD = 512
NOUT = 100
NPC = N // NCORES          # 1250 nodes per core
NCH = 11                   # dst bins per core (<=128 nodes, <=2048 edges)
NB = 2                     # blocks of 1024 edge slots per bin
BTOT = NCH * NB            # 22
NPAD = NCH * 128           # 1408 table rows per core
NROWS = NCORES * NPAD      # 11264
ROWB = 640                 # table row: 512 h bf16 + 2 z-as-bf16 + 126 pad
NBINS = 64
NSTAGES = 4
K1 = N // 2
K2 = N // 4
ZRO = NPAD + 1024          # 2432: zm2 + ro1 payload rows per core
BIG = 1e30


# ---------------------------------------------------------------------------
# host preprocessing
# ---------------------------------------------------------------------------

def _pack(x, edge_src, edge_dst, edge_weight):
    src = np.asarray(edge_src, np.int64)
    dst = np.asarray(edge_dst, np.int64)
    w = np.asarray(edge_weight, np.float32)
    x_bf = np.ascontiguousarray(np.asarray(x, np.float32)).astype(BFNP)

    # pass 1: per-core greedy bin boundaries + node->table-row map
    binrow = np.zeros((NCORES, NPC), np.int64)   # local node -> row in [0,NPAD)
    starts_all, counts_all = [], []
    for c in range(NCORES):
        lo = c * NPC
        m = (dst >= lo) & (dst < lo + NPC)
        ed = dst[m] - lo
        deg = np.bincount(ed, minlength=NPC)
        bstart, bnodes = [], []
        n0 = 0
        while n0 < NPC:
            e_acc, nn = 0, 0
            while n0 + nn < NPC and nn < 128 and e_acc + deg[n0 + nn] <= 2048:
                e_acc += deg[n0 + nn]
                nn += 1
            assert nn > 0
            bstart.append(n0)
            bnodes.append(nn)
            n0 += nn
        assert len(bstart) <= NCH, f"core {c} needs {len(bstart)} bins"
        while len(bstart) < NCH:
            bstart.append(NPC)
            bnodes.append(0)
        bstart = np.asarray(bstart, np.int64)
        bnodes = np.asarray(bnodes, np.int64)
        for b in range(NCH):
            s, nn = bstart[b], bnodes[b]
            binrow[c, s:s + nn] = b * 128 + np.arange(nn)
        starts_all.append(bstart)
        counts_all.append(bnodes)

    per_core = []
    for c in range(NCORES):
        lo = c * NPC
        m = (dst >= lo) & (dst < lo + NPC)
        es, ed, ew = src[m], dst[m] - lo, w[m]
        order = np.argsort(ed, kind="stable")
        es, ed, ew = es[order], ed[order], ew[order]
        bstart, bnodes = starts_all[c], counts_all[c]
        # edge ranges per bin (bins are consecutive node ranges)
        bin_edge_start = np.searchsorted(ed, bstart)
        bin_edge_end = np.searchsorted(ed, bstart + bnodes)

        # slot assignment (column-major within each bin's 2 blocks)
        srcslot = np.full(BTOT * 1024, -1, np.int64)
        dslot = np.zeros(BTOT * 1024, np.int64)
        wslot = np.zeros(BTOT * 1024, np.float32)
        for b in range(NCH):
            e0, e1 = bin_edge_start[b], bin_edge_end[b]
            cnt = e1 - e0
            assert cnt <= NB * 1024
            u = np.arange(cnt)
            blk = b * NB + u // 1024
            u2 = u % 1024
            pos = blk * 1024 + u2
            srcslot[pos] = es[e0:e1]
            dslot[pos] = ed[e0:e1] - bstart[b]
            wslot[pos] = ew[e0:e1]

        real = srcslot >= 0
        # xe: [128, BTOT*8*256] pre-gathered source rows, bf16
        rows = np.zeros((BTOT * 1024, FIN), BFNP)
        rows[real] = x_bf[srcslot[real]]
        xe = np.ascontiguousarray(
            rows.reshape(BTOT, 8, 128, FIN).transpose(2, 0, 1, 3)
            .reshape(128, BTOT * 8 * FIN))

        # woh: [128, BTOT*8*128] edge-weighted one-hots, bf16
        woh = np.zeros((128, BTOT * 8 * 128), np.float32)
        pos = np.nonzero(real)[0]
        blk = pos // 1024
        u2 = pos % 1024
        p = u2 % 128
        j = u2 // 128
        woh[p, (blk * 8 + j) * 128 + dslot[pos]] = wslot[pos]
        woh = woh.astype(BFNP)

        # idx2: slot -> row in the AllGather'd table
        sc = srcslot // NPC
        slo = srcslot - sc * NPC
        idx2 = np.zeros(BTOT * 1024, np.int64)
        idx2[real] = sc[real] * NPAD + binrow[sc[real], slo[real]]
        per_core.append(dict(xe=xe, woh=woh, idx2=idx2,
                             bstart=bstart, bnodes=bnodes))
    return per_core


def _wrap16(idx_flat):
    """[BTOT*1024] -> [128, BTOT*64] int16, per-block wrapped-16 replicated."""
    out = np.zeros((128, BTOT * 64), np.int16)
    for blk in range(BTOT):
        b = idx_flat[blk * 1024:(blk + 1) * 1024].astype(np.int16)
        t = b.reshape(64, 16).T          # [16, 64]
        out[:, blk * 64:(blk + 1) * 64] = np.tile(t, (8, 1))
    return out


def _host_inputs(inputs, prep):
    x = np.ascontiguousarray(np.asarray(inputs["x"], np.float32))

    def padT(a, rows, cols):
        out = np.zeros((rows, cols), np.float32)
        t = np.asarray(a, np.float32).T
        out[: t.shape[0], : t.shape[1]] = t
        return out.astype(BFNP)

    w1relT = padT(inputs["W1_rel"], FIN, HPAD)
    w1rootT = padT(inputs["W1_root"], FIN, HPAD)
    w2relT = padT(inputs["W2_rel"], HPAD, HPAD)
    w2rootT = padT(inputs["W2_root"], HPAD, HPAD)

    def rowv(v):
        out = np.zeros((1, HPAD), np.float32)
        vv = np.asarray(v, np.float32)
        out[0, : vv.shape[0]] = vv
        return out

    b1row = rowv(inputs["b1"]).astype(BFNP)
    b2row = rowv(inputs["b2"]).astype(BFNP)
    p1f = rowv(inputs["p1_w"])
    p2f = rowv(inputs["p2_w"])
    p1r = np.tile(p1f, (128, 1)).astype(BFNP)
    p2r = np.tile(p2f, (128, 1)).astype(BFNP)

    iotaB = np.tile(np.arange(NBINS, dtype=np.float32)[None, :], (128, 1))
    identb = np.eye(128, dtype=np.float32).astype(BFNP)
    identf = np.eye(128, dtype=np.float32)
    ones128f = np.ones((1, 128), np.float32)
    ones1b = np.ones((1, 128), np.float32).astype(BFNP)
    onesPb = np.ones((128, 1), np.float32).astype(BFNP)

    # mask of z positions inside the flattened z2ro AllGather payload
    g = np.arange(128 * (NCORES * ZRO // 128), dtype=np.int64)
    romask = ((g % ZRO) < NPAD).astype(np.float32).reshape(
        128, NCORES * ZRO // 128)

    lin1W = np.asarray(inputs["lin1_W"], np.float32)   # [2000, 1000]
    lin2W = np.asarray(inputs["lin2_W"], np.float32)   # [4000, 2000]
    lin3W = np.asarray(inputs["lin3_W"], np.float32)   # [100, 4000]
    lin1b = np.asarray(inputs["lin1_b"], np.float32)
    lin2b = np.asarray(inputs["lin2_b"], np.float32)
    lin3b = np.asarray(inputs["lin3_b"], np.float32)

    # lin1 replicated: rows = z layout [max 0:500 | pad | mean 512:1012 | pad]
    l1T = np.zeros((1024, 2048), np.float32)
    sh = lin1W.T                                       # [1000, 2000]
    l1T[:500, :2000] = sh[:500]
    l1T[512:1012, :2000] = sh[500:]
    l1T = l1T.astype(BFNP)
    b1h = np.zeros((128, 16), np.float32)
    b1h.T.flat[:2000] = lin1b

    per_core = []
    for c in range(NCORES):
        pr = prep[c]
        bstart, bnodes = pr["bstart"], pr["bnodes"]

        xT = np.zeros((FIN, NPAD), np.float32)
        padmask = np.zeros((128, NCH), np.float32)
        for b in range(NCH):
            s, nn = bstart[b], bnodes[b]
            if nn:
                xT[:, b * 128: b * 128 + nn] = x[c * NPC + s: c * NPC + s + nn].T
                padmask[:nn, b] = 1.0
        xT = xT.astype(BFNP)

        l2T = np.zeros((2048, 500), np.float32)
        l2T[:2000] = lin2W[c * 500:(c + 1) * 500].T
        l2T = l2T.astype(BFNP)
        b2h = np.zeros((128, 4), np.float32)
        b2h.T.flat[:500] = lin2b[c * 500:(c + 1) * 500]

        l3T = np.zeros((512, 128), np.float32)
        l3T[:500, :NOUT] = lin3W[:, c * 500:(c + 1) * 500].T
        l3T = l3T.astype(BFNP)
        b3h = np.zeros((128, 1), np.float32)
        b3h[:NOUT, 0] = lin3b

        per_core.append(dict(
            xe=pr["xe"], woh=pr["woh"], idx2=_wrap16(pr["idx2"]),
            padmask=padmask, xT=xT,
            w1relT=w1relT, w1rootT=w1rootT, w2relT=w2relT, w2rootT=w2rootT,
            b1row=b1row, b2row=b2row, p1f=p1f, p2f=p2f, p1r=p1r, p2r=p2r,
            iotaB=iotaB, identb=identb, identf=identf, ones128f=ones128f,
            ones1b=ones1b, onesPb=onesPb, romask=romask,
            l1T=l1T, b1h=b1h, l2T=l2T, b2h=b2h, l3T=l3T, b3h=b3h,
        ))
    return per_core


# ---------------------------------------------------------------------------
# device program
# ---------------------------------------------------------------------------

def _build():
    nc = bacc.Bacc("TRN2", target_bir_lowering=False, debug=False,
                   num_devices=NCORES)

    def din(name, shape, dt=FP32):
        return nc.dram_tensor(name, shape, dt, kind="ExternalInput")

    xe = din("xe", [128, BTOT * 8 * FIN], BF16)
    woh = din("woh", [128, BTOT * 8 * 128], BF16)
    idx2 = din("idx2", [128, BTOT * 64], I16)
    padmask = din("padmask", [128, NCH])
    xT = din("xT", [FIN, NPAD], BF16)
    w1relT = din("w1relT", [FIN, HPAD], BF16)
    w1rootT = din("w1rootT", [FIN, HPAD], BF16)
    w2relT = din("w2relT", [HPAD, HPAD], BF16)
    w2rootT = din("w2rootT", [HPAD, HPAD], BF16)
    b1row = din("b1row", [1, HPAD], BF16)
    b2row = din("b2row", [1, HPAD], BF16)
    p1f = din("p1f", [1, HPAD])
    p2f = din("p2f", [1, HPAD])
    p1r = din("p1r", [128, HPAD], BF16)
    p2r = din("p2r", [128, HPAD], BF16)
    iotaB = din("iotaB", [128, NBINS])
    identb = din("identb", [128, 128], BF16)
    identf = din("identf", [128, 128])
    ones128f = din("ones128f", [1, 128])
    ones1b = din("ones1b", [1, 128], BF16)
    onesPb = din("onesPb", [128, 1], BF16)
    romask = din("romask", [128, NCORES * ZRO // 128])
    l1T = din("l1T", [1024, 2048], BF16)
    b1h = din("b1h", [128, 16])
    l2T = din("l2T", [2048, 500], BF16)
    b2h = din("b2h", [128, 4])
    l3T = din("l3T", [512, 128], BF16)
    b3h = din("b3h", [128, 1])

    out = nc.dram_tensor("out", [1, NOUT], FP32, kind="ExternalOutput")

    RG = [list(range(NCORES))]

    with tile.TileContext(nc) as tc:
        with (
            tc.tile_pool(name="const", bufs=1) as cp,
            tc.tile_pool(name="stream", bufs=3) as sp,
            tc.tile_pool(name="gather", bufs=2) as gp,
            tc.tile_pool(name="work", bufs=1) as wp,
            tc.tile_pool(name="big", bufs=1) as bigp,
            tc.tile_pool(name="psA", bufs=2, space="PSUM") as psA,
            tc.tile_pool(name="psB", bufs=2, space="PSUM") as psB,
            tc.tile_pool(name="psT", bufs=2, space="PSUM") as psT,
            tc.tile_pool(name="psS", bufs=1, space="PSUM") as psS,
            tc.tile_pool(name="dram", bufs=1, space="DRAM") as dr,
        ):
            def load(src, dt=FP32, tag=None):
                tl = cp.tile(list(src.shape), dt, tag=tag or src.name)
                nc.sync.dma_start(tl[:], src[:])
                return tl

            idx2_t = load(idx2, I16)
            pad_t = load(padmask)
            iob_t = load(iotaB)
            idb_t = load(identb, BF16)
            idf_t = load(identf)
            ones_t = load(ones128f)
            ones1b_t = load(ones1b, BF16)
            onesPb_t = load(onesPb, BF16)
            b1row_t = load(b1row, BF16)
            b2row_t = load(b2row, BF16)
            p1f_t = load(p1f)
            p2f_t = load(p2f)
            p1r_t = load(p1r, BF16)
            p2r_t = load(p2r, BF16)
            rom_t = load(romask)

            def load_chunks(src, nchunks, cols, tag, dt=BF16):
                ts = []
                for k in range(nchunks):
                    t = cp.tile([128, cols], dt, tag=f"{tag}{k}")
                    nc.sync.dma_start(t[:], src[k * 128:(k + 1) * 128, :cols])
                    ts.append(t)
                return ts

            w1rel_t = load_chunks(w1relT, 2, HPAD, "w1rel")
            w1root_t = load_chunks(w1rootT, 2, HPAD, "w1root")
            w2rel_t = load_chunks(w2relT, 4, HPAD, "w2rel")
            w2root_t = load_chunks(w2rootT, 4, HPAD, "w2root")
            xT_t = load_chunks(xT, 2, NPAD, "xTc")

            # DRAM internal tiles
            tbl = dr.tile([NPAD, ROWB], BF16)
            tblag = dr.tile([NROWS, ROWB], BF16, addr_space="Shared")
            zsh1 = dr.tile([NPAD, 1], FP32)
            zag1 = dr.tile([NROWS, 1], FP32, addr_space="Shared")
            z2ro = dr.tile([ZRO, 1], FP32)
            z2roag = dr.tile([NCORES * ZRO, 1], FP32, addr_space="Shared")
            ro2in = dr.tile([2, HPAD], FP32)
            ro2ag = dr.tile([2 * NCORES, HPAD], FP32, addr_space="Shared")
            oin = dr.tile([128, 1], FP32)
            oar = dr.tile([128, 1], FP32, addr_space="Shared")

            # -------- norms first (Sqrt table load hides under L1) ---------
            def inv_norm_b(pf_t, lname):
                """[128,1] broadcast of 1/||p||."""
                sq = wp.tile([1, HPAD], FP32, tag="pnsq")
                nc.vector.tensor_tensor(out=sq[:], in0=pf_t[:], in1=pf_t[:],
                                        op=OP.mult)
                n2 = wp.tile([1, 1], FP32, tag="pn2")
                nc.vector.tensor_reduce(out=n2[:], in_=sq[:], op=OP.add,
                                        axis=AX.X)
                nc.scalar.activation(n2[:], n2[:], ACT.Sqrt)
                nc.vector.reciprocal(n2[:], n2[:])
                ib_ps = psS.tile([128, 1], FP32, tag="small")
                nc.tensor.matmul(out=ib_ps[:], lhsT=ones_t[:], rhs=n2[:],
                                 start=True, stop=True)
                ib = wp.tile([128, 1], FP32, tag=f"invbs{lname}")
                nc.vector.tensor_copy(ib[:], ib_ps[:])
                return ib

            inv1b = inv_norm_b(p1f_t, "l1")
            inv2b = inv_norm_b(p2f_t, "l2")

            # ---------------- histogram k-th threshold ---------------------
            NF2 = NCORES * ZRO // 128
            S_big = wp.tile([128, NF2 * NBINS], BF16, tag="Sbig")

            def topk_tau(zt, nfree, k, lname):
                """zt: [128, nfree] fp32 scores (pads/masked = -BIG).
                returns [128,1] tile with the k-th-largest threshold."""
                mm = wp.tile([128, 2], FP32, tag="mm")
                msk = wp.tile([128, nfree], FP32, tag=f"hmsk{lname}")
                nc.vector.tensor_scalar(msk[:], zt[:], -1e29, 2e30, OP.is_lt,
                                        OP.mult)
                nc.vector.tensor_tensor(out=msk[:], in0=msk[:], in1=zt[:],
                                        op=OP.add)
                nc.vector.tensor_reduce(out=mm[:, 0:1], in_=msk[:], op=OP.min,
                                        axis=AX.X)
                nc.vector.tensor_reduce(out=mm[:, 1:2], in_=zt[:], op=OP.max,
                                        axis=AX.X)
                ztb = wp.tile([128, nfree], BF16, tag=f"ztb{lname}")
                nc.vector.tensor_copy(ztb[:], zt[:])
                lw = wp.tile([1, 2], FP32, tag="lw")  # [lo, w]
                mmT = wp.tile([1, 2, 128], FP32, tag="mmTs")
                for col in range(2):
                    mmT_ps = psS.tile([1, 128], FP32, tag="small")
                    nc.tensor.transpose(out=mmT_ps[:], in_=mm[:, col:col + 1],
                                        identity=idf_t[:])
                    nc.vector.tensor_copy(mmT[:, col, :], mmT_ps[:])
                nc.vector.tensor_reduce(out=lw[:, 0:1], in_=mmT[:, 0, :],
                                        op=OP.min, axis=AX.X)
                nc.vector.tensor_reduce(out=lw[:, 1:2], in_=mmT[:, 1, :],
                                        op=OP.max, axis=AX.X)
                nc.vector.tensor_scalar_add(lw[:, 0:1], lw[:, 0:1], -1e-3)
                nc.vector.tensor_scalar_add(lw[:, 1:2], lw[:, 1:2], 1e-3)
                nc.vector.tensor_tensor(out=lw[:, 1:2], in0=lw[:, 1:2],
                                        in1=lw[:, 0:1], op=OP.subtract)
                nc.vector.tensor_scalar_mul(lw[:, 1:2], lw[:, 1:2], 1.0 / NBINS)

                for st in range(NSTAGES):
                    lwb_ps = psS.tile([128, 2], FP32, tag="small")
                    nc.tensor.matmul(out=lwb_ps[:], lhsT=ones_t[:], rhs=lw[:],
                                     start=True, stop=True)
                    lwb = wp.tile([128, 2], FP32, tag="lwbs")
                    nc.vector.tensor_copy(lwb[:], lwb_ps[:])
                    tt = wp.tile([128, NBINS], FP32, tag="tt")
                    nc.vector.tensor_scalar(tt[:], iob_t[:], lwb[:, 1:2],
                                            lwb[:, 0:1], OP.mult, OP.add)
                    ttb = wp.tile([128, NBINS], BF16, tag="ttb")
                    nc.vector.tensor_copy(ttb[:], tt[:])
                    S = S_big[:, :nfree * NBINS].rearrange(
                        "p (n j) -> p n j", j=NBINS)
                    nc.vector.tensor_tensor(
                        out=S,
                        in0=ztb[:].unsqueeze(2).broadcast_to(
                            [128, nfree, NBINS]),
                        in1=ttb[:].unsqueeze(1).broadcast_to(
                            [128, nfree, NBINS]),
                        op=OP.is_ge)
                    cntp = wp.tile([128, NBINS], BF16, tag="cntp")
                    with nc.allow_low_precision(
                            reason="counts <= nfree are exact in bf16"):
                        nc.vector.tensor_reduce(
                            out=cntp[:], in_=S.rearrange("p n j -> p j n"),
                            op=OP.add, axis=AX.X)
                    cnt_ps = psS.tile([1, NBINS], FP32, tag="small")
                    nc.tensor.matmul(out=cnt_ps[:], lhsT=onesPb_t[:],
                                     rhs=cntp[:], start=True, stop=True)
                    fl = wp.tile([1, NBINS], FP32, tag="fl")
                    nc.vector.tensor_scalar(fl[:], cnt_ps[:], float(k), None,
                                            OP.is_ge)
                    js = wp.tile([1, 1], FP32, tag="js")
                    nc.vector.tensor_reduce(out=js[:], in_=fl[:], op=OP.add,
                                            axis=AX.X)
                    nc.vector.tensor_scalar_add(js[:], js[:], -1.0)
                    nc.vector.tensor_scalar(lw[:, 0:1], js[:], lw[:, 1:2],
                                            lw[:, 0:1], OP.mult, OP.add)
                    if st != NSTAGES - 1:
                        nc.vector.tensor_scalar_mul(lw[:, 1:2], lw[:, 1:2],
                                                    1.0 / NBINS)
                taub_ps = psS.tile([128, 1], FP32, tag="small")
                nc.tensor.matmul(out=taub_ps[:], lhsT=ones_t[:],
                                 rhs=lw[:, 0:1], start=True, stop=True)
                taub = wp.tile([128, 1], FP32, tag=f"taubs{lname}")
                nc.vector.tensor_copy(taub[:], taub_ps[:])
                return taub

            # ======================= layer 1 ===============================
            h1 = bigp.tile([128, NCH * HPAD], BF16, tag="h1_all")
            z1 = wp.tile([128, NCH], FP32, tag="z1")
            aggT1 = [bigp.tile([128, NPAD], BF16, tag=f"aggT1_{fc}",
                               name=f"aggT1_{fc}")
                     for fc in range(2)]

            for b in range(NCH):
                agg_ps = psA.tile([128, HPAD], FP32, tag="aggps")
                for k in range(NB):
                    B = b * NB + k
                    xeb = sp.tile([128, 8, FIN], BF16, tag="xeb")
                    nc.sync.dma_start(
                        xeb[:].rearrange("p a f -> p (a f)"),
                        xe[:, B * 8 * FIN:(B + 1) * 8 * FIN])
                    wohb = sp.tile([128, 8, 128], BF16, tag="wohb1")
                    nc.sync.dma_start(
                        wohb[:].rearrange("p a d -> p (a d)"),
                        woh[:, B * 1024:(B + 1) * 1024])
                    for j in range(8):
                        nc.tensor.matmul(
                            out=agg_ps[:, :FIN], lhsT=wohb[:, j, :],
                            rhs=xeb[:, j, :],
                            start=(k == 0 and j == 0),
                            stop=(k == NB - 1 and j == 7))
                # transpose agg -> aggT1 chunks
                aggc = wp.tile([128, FIN], BF16, tag="aggc", bufs=2)
                nc.scalar.activation(aggc[:], agg_ps[:, :FIN], ACT.Copy)
                for fc in range(2):
                    tps = psT.tile([128, 128], BF16, tag="tps")
                    nc.tensor.transpose(out=tps[:],
                                        in_=aggc[:, fc * 128:(fc + 1) * 128],
                                        identity=idb_t[:])
                    nc.scalar.activation(aggT1[fc][:, b * 128:(b + 1) * 128],
                                         tps[:], ACT.Copy)
                # dense: h = relu(b1 + aggT.T @ w1relT + xT.T @ w1rootT)
                hp = psB.tile([128, HPAD], FP32, tag="hps")
                nc.tensor.matmul(out=hp[:], lhsT=ones1b_t[:], rhs=b1row_t[:],
                                 start=True, stop=False)
                for fc in range(2):
                    nc.tensor.matmul(
                        out=hp[:], lhsT=aggT1[fc][:, b * 128:(b + 1) * 128],
                        rhs=w1rel_t[fc][:], start=False, stop=False)
                for fc in range(2):
                    nc.tensor.matmul(
                        out=hp[:], lhsT=xT_t[fc][:, b * 128:(b + 1) * 128],
                        rhs=w1root_t[fc][:], start=False, stop=(fc == 1))
                hc = h1[:, b * HPAD:(b + 1) * HPAD]
                nc.scalar.activation(hc, hp[:], ACT.Relu)
                # z score (fp32)
                scr = wp.tile([128, HPAD], FP32, tag="scr", bufs=2)
                nc.vector.tensor_tensor(out=scr[:], in0=hc, in1=p1r_t[:],
                                        op=OP.mult)
                nc.vector.tensor_reduce(out=z1[:, b:b + 1], in_=scr[:],
                                        op=OP.add, axis=AX.X)
                # table row: [h | z | pad]
                tblb = wp.tile([128, ROWB], BF16, tag="tblb", bufs=2)
                nc.scalar.activation(tblb[:, 0:HPAD], hp[:], ACT.Relu)
                nc.vector.tensor_copy(
                    tblb[:, HPAD:HPAD + 2].bitcast(FP32), z1[:, b:b + 1])
                nc.sync.dma_start(tbl[b * 128:(b + 1) * 128, :], tblb[:])

            # masked z for selection
            pm30 = wp.tile([128, NCH], FP32, tag="pm30")
            nc.vector.tensor_scalar(pm30[:], pad_t[:], 1.0, BIG, OP.subtract,
                                    OP.mult)
            zm1 = wp.tile([128, NCH], FP32, tag="zm1")
            nc.vector.tensor_tensor(out=zm1[:], in0=z1[:], in1=pad_t[:],
                                    op=OP.mult)
            nc.vector.tensor_tensor(out=zm1[:], in0=zm1[:], in1=pm30[:],
                                    op=OP.add)
            nc.sync.dma_start(
                zsh1[:].rearrange("(b p) o -> p (b o)", p=128), zm1[:])
            nc.gpsimd.collective_compute(
                "AllGather", OP.bypass, replica_groups=RG,
                ins=[zsh1[:]], outs=[zag1[:]])
            nc.gpsimd.collective_compute(
                "AllGather", OP.bypass, replica_groups=RG,
                ins=[tbl[:]], outs=[tblag[:]])

            zt1 = wp.tile([128, NROWS // 128], FP32, tag="zt1")
            nc.sync.dma_start(
                zt1[:], zag1[:].rearrange("(p f) o -> p (f o)", p=128))
            tau1b = topk_tau(zt1, NROWS // 128, K1, "l1")

            # a1 per local bin + kept masks
            kp1 = wp.tile([128, NCH], FP32, tag="kp1")
            nc.vector.tensor_scalar(kp1[:], zm1[:], tau1b[:, 0:1], None,
                                    OP.is_ge)
            s1 = wp.tile([128, NCH], FP32, tag="s1")
            nc.scalar.activation(s1[:], z1[:], ACT.Tanh, scale=inv1b[:, 0:1])
            a1 = wp.tile([128, NCH], FP32, tag="a1")
            nc.vector.tensor_tensor(out=a1[:], in0=s1[:], in1=kp1[:],
                                    op=OP.mult)
            km30 = wp.tile([128, NCH], FP32, tag="km30")
            nc.vector.tensor_scalar(km30[:], kp1[:], 1.0, BIG, OP.subtract,
                                    OP.mult)

            # g1 (scaled, masked transpose) + readout 1
            gmT1 = [bigp.tile([128, NPAD], BF16, tag=f"gmT1_{fc}",
                              name=f"gmT1_{fc}")
                    for fc in range(4)]
            ro1s_ps = psS.tile([1, HPAD], FP32, tag="rosum")
            for b in range(NCH):
                hc = h1[:, b * HPAD:(b + 1) * HPAD]
                g1c = wp.tile([128, HPAD], BF16, tag="g1c", bufs=2)
                nc.vector.tensor_scalar(g1c[:], hc, a1[:, b:b + 1], None,
                                        OP.mult)
                nc.tensor.matmul(out=ro1s_ps[:], lhsT=onesPb_t[:], rhs=g1c[:],
                                 start=(b == 0), stop=(b == NCH - 1))
                gmc = wp.tile([128, HPAD], BF16, tag="gmc", bufs=2)
                nc.vector.tensor_scalar(gmc[:], hc, a1[:, b:b + 1],
                                        km30[:, b:b + 1], OP.mult, OP.add)
                for fc in range(4):
                    tps = psT.tile([128, 128], BF16, tag="tps")
                    nc.tensor.transpose(out=tps[:],
                                        in_=gmc[:, fc * 128:(fc + 1) * 128],
                                        identity=idb_t[:])
                    nc.scalar.activation(gmT1[fc][:, b * 128:(b + 1) * 128],
                                         tps[:], ACT.Copy)
            m1T = wp.tile([128, 4], FP32, tag="m1T")
            for fc in range(4):
                nc.vector.tensor_reduce(out=m1T[:, fc:fc + 1], in_=gmT1[fc][:],
                                        op=OP.max, axis=AX.X)
            ro1s = wp.tile([1, HPAD], FP32, tag="ro1s")
            nc.vector.tensor_copy(ro1s[:], ro1s_ps[:])
            # ro1 rides in the z2ro payload (rows NPAD.. and NPAD+512..)
            nc.sync.dma_start(z2ro[NPAD:NPAD + HPAD, :]
                              .rearrange("f o -> o f"), ro1s[:])
            nc.sync.dma_start(
                z2ro[NPAD + HPAD:NPAD + 1024, :]
                .rearrange("(c p) o -> p (c o)", p=128), m1T[:])

            # ======================= layer 2 ===============================
            h2 = bigp.tile([128, NCH * HPAD], BF16, tag="h2_all")
            z2 = wp.tile([128, NCH], FP32, tag="z2")
            aggT2 = [bigp.tile([128, NPAD], BF16, tag=f"aggT2_{fc}",
                               name=f"aggT2_{fc}")
                     for fc in range(4)]

            for b in range(NCH):
                agg_ps = psA.tile([128, HPAD], FP32, tag="aggps")
                for k in range(NB):
                    B = b * NB + k
                    gt = gp.tile([128, 8, ROWB], BF16, tag="gath")
                    nc.gpsimd.dma_gather(
                        gt[:], tblag[:], idx2_t[:, B * 64:(B + 1) * 64],
                        1024, 1024, ROWB)
                    wohb = sp.tile([128, 8, 128], BF16, tag="wohb2")
                    nc.sync.dma_start(
                        wohb[:].rearrange("p a d -> p (a d)"),
                        woh[:, B * 1024:(B + 1) * 1024])
                    # per-slot scale a1 = tanh(z*inv)*(z>=tau)
                    zg = gt[:, :, HPAD:HPAD + 2].bitcast(FP32) \
                        .rearrange("p a o -> p (a o)")
                    kp8 = wp.tile([128, 8], FP32, tag="kp8", bufs=2)
                    nc.vector.tensor_scalar(kp8[:], zg, tau1b[:, 0:1], None,
                                            OP.is_ge)
                    s8 = wp.tile([128, 8], FP32, tag="s8", bufs=2)
                    nc.scalar.activation(s8[:], zg, ACT.Tanh,
                                         scale=inv1b[:, 0:1])
                    a1s = wp.tile([128, 8], BF16, tag="a1s", bufs=2)
                    nc.vector.tensor_tensor(out=a1s[:], in0=s8[:], in1=kp8[:],
                                            op=OP.mult)
                    ohs = wp.tile([128, 8, 128], BF16, tag="ohs", bufs=2)
                    nc.vector.tensor_tensor(
                        out=ohs[:], in0=wohb[:],
                        in1=a1s[:].unsqueeze(2).broadcast_to([128, 8, 128]),
                        op=OP.mult)
                    for j in range(8):
                        nc.tensor.matmul(
                            out=agg_ps[:], lhsT=ohs[:, j, :],
                            rhs=gt[:, j, 0:HPAD],
                            start=(k == 0 and j == 0),
                            stop=(k == NB - 1 and j == 7))
                aggc = wp.tile([128, HPAD], BF16, tag="aggc2", bufs=2)
                nc.scalar.activation(aggc[:], agg_ps[:], ACT.Copy)
                for fc in range(4):
                    tps = psT.tile([128, 128], BF16, tag="tps")
                    nc.tensor.transpose(out=tps[:],
                                        in_=aggc[:, fc * 128:(fc + 1) * 128],
                                        identity=idb_t[:])
                    nc.scalar.activation(aggT2[fc][:, b * 128:(b + 1) * 128],
                                         tps[:], ACT.Copy)
                hp = psB.tile([128, HPAD], FP32, tag="hps")
                nc.tensor.matmul(out=hp[:], lhsT=ones1b_t[:], rhs=b2row_t[:],
                                 start=True, stop=False)
                for fc in range(4):
                    nc.tensor.matmul(
                        out=hp[:], lhsT=aggT2[fc][:, b * 128:(b + 1) * 128],
                        rhs=w2rel_t[fc][:], start=False, stop=False)
                for fc in range(4):
                    nc.tensor.matmul(
                        out=hp[:], lhsT=gmT1[fc][:, b * 128:(b + 1) * 128],
                        rhs=w2root_t[fc][:], start=False, stop=(fc == 3))
                hc = h2[:, b * HPAD:(b + 1) * HPAD]
                nc.scalar.activation(hc, hp[:], ACT.Relu)
                scr = wp.tile([128, HPAD], FP32, tag="scr", bufs=2)
                nc.vector.tensor_tensor(out=scr[:], in0=hc, in1=p2r_t[:],
                                        op=OP.mult)
                nc.vector.tensor_reduce(out=z2[:, b:b + 1], in_=scr[:],
                                        op=OP.add, axis=AX.X)

            # masked z2 (kept-in-l1 only) -> z2ro payload -> AllGather
            zm2 = wp.tile([128, NCH], FP32, tag="zm2")
            nc.vector.tensor_tensor(out=zm2[:], in0=z2[:], in1=kp1[:],
                                    op=OP.mult)
            nc.vector.tensor_tensor(out=zm2[:], in0=zm2[:], in1=km30[:],
                                    op=OP.add)
            nc.sync.dma_start(
                z2ro[0:NPAD, :].rearrange("(b p) o -> p (b o)", p=128),
                zm2[:])
            nc.gpsimd.collective_compute(
                "AllGather", OP.bypass, replica_groups=RG,
                ins=[z2ro[:]], outs=[z2roag[:]])

            # tau2 over the masked flat payload
            ztr = wp.tile([128, NF2], FP32, tag="ztr")
            nc.sync.dma_start(
                ztr[:], z2roag[:].rearrange("(p f) o -> p (f o)", p=128))
            zt2 = wp.tile([128, NF2], FP32, tag="zt2")
            nc.vector.tensor_tensor(out=zt2[:], in0=ztr[:], in1=rom_t[:],
                                    op=OP.mult)
            rm30 = wp.tile([128, NF2], FP32, tag="rm30")
            nc.vector.tensor_scalar(rm30[:], rom_t[:], 1.0, BIG, OP.subtract,
                                    OP.mult)
            nc.vector.tensor_tensor(out=zt2[:], in0=zt2[:], in1=rm30[:],
                                    op=OP.add)
            tau2b = topk_tau(zt2, NF2, K2, "l2")

            kp2 = wp.tile([128, NCH], FP32, tag="kp2")
            nc.vector.tensor_scalar(kp2[:], zm2[:], tau2b[:, 0:1], None,
                                    OP.is_ge)
            s2 = wp.tile([128, NCH], FP32, tag="s2")
            nc.scalar.activation(s2[:], z2[:], ACT.Tanh, scale=inv2b[:, 0:1])
            a2 = wp.tile([128, NCH], FP32, tag="a2")
            nc.vector.tensor_tensor(out=a2[:], in0=s2[:], in1=kp2[:],
                                    op=OP.mult)
            km30b = wp.tile([128, NCH], FP32, tag="km30b")
            nc.vector.tensor_scalar(km30b[:], kp2[:], 1.0, BIG, OP.subtract,
                                    OP.mult)

            ro2s_ps = psS.tile([1, HPAD], FP32, tag="rosum")
            m2T = wp.tile([128, 4], FP32, tag="m2T")
            nc.vector.memset(m2T[:], -1e30)
            for b in range(NCH):
                hc = h2[:, b * HPAD:(b + 1) * HPAD]
                g2c = wp.tile([128, HPAD], BF16, tag="g1c", bufs=2)
                nc.vector.tensor_scalar(g2c[:], hc, a2[:, b:b + 1], None,
                                        OP.mult)
                nc.tensor.matmul(out=ro2s_ps[:], lhsT=onesPb_t[:], rhs=g2c[:],
                                 start=(b == 0), stop=(b == NCH - 1))
                gmc = wp.tile([128, HPAD], BF16, tag="gmc", bufs=2)
                nc.vector.tensor_scalar(gmc[:], hc, a2[:, b:b + 1],
                                        km30b[:, b:b + 1], OP.mult, OP.add)
                for fc in range(4):
                    tps = psT.tile([128, 128], BF16, tag="tps")
                    nc.tensor.transpose(out=tps[:],
                                        in_=gmc[:, fc * 128:(fc + 1) * 128],
                                        identity=idb_t[:])
                    red = wp.tile([128, 1], FP32, tag="redm", bufs=2)
                    nc.vector.tensor_reduce(out=red[:], in_=tps[:],
                                            op=OP.max, axis=AX.X)
                    nc.vector.tensor_tensor(out=m2T[:, fc:fc + 1],
                                            in0=m2T[:, fc:fc + 1],
                                            in1=red[:], op=OP.max)
            ro2s = wp.tile([1, HPAD], FP32, tag="ro2s")
            nc.vector.tensor_copy(ro2s[:], ro2s_ps[:])
            nc.sync.dma_start(ro2in[0:1, :], ro2s[:])
            nc.sync.dma_start(
                ro2in[1:2, :].rearrange("o (c p) -> p (o c)", p=128), m2T[:])
            nc.gpsimd.collective_compute(
                "AllGather", OP.bypass, replica_groups=RG,
                ins=[ro2in[:]], outs=[ro2ag[:]])

            # ======================= readout combine + head ================
            # ro1 lives in z2roag rows [s*ZRO+NPAD, s*ZRO+NPAD+1024)
            mx1 = wp.tile([128, 4], FP32, tag="mx1")
            mn1 = wp.tile([128, 4], FP32, tag="mn1")
            sums1 = wp.tile([128, 4, NCORES], FP32, tag="cmb1")
            maxs1 = wp.tile([128, 4, NCORES], FP32, tag="cmbm1")
            for s in range(NCORES):
                base = s * ZRO + NPAD
                nc.sync.dma_start(
                    sums1[:, :, s],
                    z2roag[base:base + HPAD, :]
                    .rearrange("(c p) o -> p (c o)", p=128))
                nc.sync.dma_start(
                    maxs1[:, :, s],
                    z2roag[base + HPAD:base + 1024, :]
                    .rearrange("(c p) o -> p (c o)", p=128))
            nc.vector.tensor_reduce(out=mn1[:], in_=sums1[:], op=OP.add,
                                    axis=AX.X)
            nc.vector.tensor_reduce(out=mx1[:], in_=maxs1[:], op=OP.max,
                                    axis=AX.X)
            nc.vector.tensor_scalar_mul(mn1[:], mn1[:], 1.0 / K1)

            mx2 = wp.tile([128, 4], FP32, tag="mx2")
            mn2 = wp.tile([128, 4], FP32, tag="mn2")
            sums2 = wp.tile([128, 4, 2 * NCORES], FP32, tag="cmb2")
            for r in range(2 * NCORES):
                nc.sync.dma_start(
                    sums2[:, :, r],
                    ro2ag[r:r + 1, :].rearrange("o (c p) -> p (o c)", p=128))
            s_ap = sums2[:].rearrange("p c (s t) -> p c t s", t=2)
            nc.vector.tensor_reduce(out=mn2[:], in_=s_ap[:, :, 0, :],
                                    op=OP.add, axis=AX.X)
            nc.vector.tensor_reduce(out=mx2[:], in_=s_ap[:, :, 1, :],
                                    op=OP.max, axis=AX.X)
            nc.vector.tensor_scalar_mul(mn2[:], mn2[:], 1.0 / K2)

            zT = wp.tile([128, 8], FP32, tag="zT")
            nc.vector.tensor_tensor(out=zT[:, 0:4], in0=mx1[:], in1=mx2[:],
                                    op=OP.add)
            nc.vector.tensor_tensor(out=zT[:, 4:8], in0=mn1[:], in1=mn2[:],
                                    op=OP.add)
            zTb = wp.tile([128, 8], BF16, tag="zTb")
            nc.vector.tensor_copy(zTb[:], zT[:])

            # lin1 replicated: z1hT [128, 16]
            b1h_t = load(b1h)
            z1hT = wp.tile([128, 16], BF16, tag="z1hT")
            o1p = psB.tile([128, 16], FP32, tag="hps")
            for t in range(8):
                l1c = sp.tile([128, 2048], BF16, tag="l1s")
                nc.sync.dma_start(l1c[:], l1T[t * 128:(t + 1) * 128, :])
                for m in range(16):
                    nc.tensor.matmul(
                        out=o1p[:, m:m + 1],
                        lhsT=l1c[:, m * 128:(m + 1) * 128],
                        rhs=zTb[:, t:t + 1],
                        start=(t == 0 and m == 0), stop=(t == 7 and m == 15))
            for m in range(16):
                nc.scalar.activation(z1hT[:, m:m + 1], o1p[:, m:m + 1],
                                     ACT.Relu, bias=b1h_t[:, m:m + 1])

            # lin2 shard: z2hT [128, 4] (500 rows via msl slices)
            b2h_t = load(b2h)
            z2hT = wp.tile([128, 4], BF16, tag="z2hT")
            nc.vector.memset(z2hT[:], 0.0)
            msl = [(0, 128), (128, 256), (256, 384), (384, 500)]
            o2p = psB.tile([128, 4], FP32, tag="hps")
            for t in range(16):
                l2c = sp.tile([128, 500], BF16, tag="l2s")
                nc.sync.dma_start(l2c[:], l2T[t * 128:(t + 1) * 128, :])
                for mi, (m0, m1) in enumerate(msl):
                    nc.tensor.matmul(out=o2p[: m1 - m0, mi:mi + 1],
                                     lhsT=l2c[:, m0:m1],
                                     rhs=z1hT[:, t:t + 1],
                                     start=(t == 0 and mi == 0),
                                     stop=(t == 15 and mi == 3))
            for mi, (m0, m1) in enumerate(msl):
                nc.scalar.activation(z2hT[: m1 - m0, mi:mi + 1],
                                     o2p[: m1 - m0, mi:mi + 1],
                                     ACT.Relu, bias=b2h_t[: m1 - m0, mi:mi + 1])

            # lin3 partial (own contraction shard) + AllReduce
            l3_t = load_chunks(l3T, 4, 128, "l3Tc")
            b3h_t = load(b3h)
            o3p = psB.tile([128, 1], FP32, tag="hps")
            for t in range(4):
                nc.tensor.matmul(out=o3p[:], lhsT=l3_t[t][:],
                                 rhs=z2hT[:, t:t + 1],
                                 start=(t == 0), stop=(t == 3))
            o3 = wp.tile([128, 1], FP32, tag="o3")
            nc.vector.tensor_copy(o3[:], o3p[:])
            nc.sync.dma_start(oin[:], o3[:])
            nc.gpsimd.collective_compute(
                "AllReduce", OP.add, replica_groups=RG,
                ins=[oin[:]], outs=[oar[:]])
            fin = wp.tile([128, 1], FP32, tag="fin")
            nc.sync.dma_start(fin[:], oar[:])
            nc.scalar.activation(fin[:], fin[:], ACT.Sigmoid,
                                 bias=b3h_t[:, 0:1])
            nc.sync.dma_start(out[:].rearrange("o f -> f o"), fin[:NOUT, :])

    nc.compile()
    return nc


# ---------------------------------------------------------------------------
# entry point
# ---------------------------------------------------------------------------

_CACHE = {}
TRACE = False


def kernel(**inputs):
    prep = _pack(inputs["x"], inputs["edge_src"], inputs["edge_dst"],
                 inputs["edge_weight"])
    if "nc" not in _CACHE:
        _CACHE["nc"] = _build()
    nc = _CACHE["nc"]
    in_maps = _host_inputs(inputs, prep)
    res = bass_utils.run_bass_kernel_spmd(
        nc, in_maps, core_ids=list(range(NCORES)), trace=TRACE)
    kernel.last_results = res
    return res.results[0]["out"]


if __name__ == "__main__":
    dat = np.load("/tmp/inputs.npz")
    inputs = {k: dat[k] for k in dat.files}
    got = kernel(**inputs)
    exp = np.load("/tmp/expected.npy")
    err = np.abs(got - exp).max()
    rel = err / np.abs(exp).max()
    print("out[0,:6] =", got[0, :6])
    print("exp[0,:6] =", exp[0, :6])
    print("max abs err:", err, "rel:", rel)


# revision 27
# speedup vs baseline: 2.0186x; 1.1886x over previous
"""Trainium2 Bass kernel for nn_Net_48301202211072 (GNN message passing).

2-layer GraphConv + TopKPooling + readout + MLP head, sharded over 8
NeuronCores. Strategy (v2):

- Nodes (and incident edges, grouped by destination) are sharded across
  cores. Edges are packed column-major into 11 destination bins per core
  (<=128 nodes and <=2048 edges per bin), 2 blocks of 1024 edge slots
  per bin. Everything is bf16 except score/threshold arithmetic.
- Layer-1 aggregation: the host pre-arranges per-edge source rows (xe)
  and edge-weighted one-hot matrices (Woh); the device streams both and
  aggregates with plain PE matmuls (no dma_gather, no per-edge DVE).
- Layer-2 aggregation: each node's table row [h1 | z1] is AllGather'd
  right after conv1 (before topk), so the big collective and the tau1
  histogram overlap; dma_gather fetches rows per edge and the per-slot
  scale tanh(z/||p||)*(z>=tau)*w is applied via the one-hot weights.
- TopK threshold: replicated 4-stage 64-bin histogram over the
  AllGather'd scores (bf16 compares, exact-enough within tolerance).
- Readout: ones-matmul mean + masked-transpose max, combined via small
  AllGathers (ro1 rides in the z2 AllGather payload).
- Head: lin1 replicated, lin2 sharded by rows, lin3 by contraction,
  one final AllReduce.
"""
import math
import sys

import numpy as np
import ml_dtypes

sys.path.insert(0, "/opt/trn_rl_repo")

import concourse.bacc as bacc  # noqa: E402
import concourse.mybir as mybir  # noqa: E402
import concourse.tile as tile  # noqa: E402
from concourse import bass_utils  # noqa: E402

FP32 = mybir.dt.float32
BF16 = mybir.dt.bfloat16
I16 = mybir.dt.int16
AX = mybir.AxisListType
OP = mybir.AluOpType
ACT = mybir.ActivationFunctionType
BFNP = ml_dtypes.bfloat16

NCORES = 8
N = 10000
FIN = 256
HID = 500
HPAD = 512
NOUT = 100
NPC = N // NCORES          # 1250 nodes per core
NCH = 11                   # dst bins per core (<=128 nodes, <=2048 edges)
NB = 2                     # blocks of 1024 edge slots per bin
BTOT = NCH * NB            # 22
NPAD = NCH * 128           # 1408 table rows per core
NROWS = NCORES * NPAD      # 11264
ROWB = 640                 # table row: 512 h bf16 + 2 z-as-bf16 + 126 pad
NBINS = 32
NSTAGES = 3
K1 = N // 2
K2 = N // 4
ZRO = NPAD + 1024          # 2432: zm2 + ro1 payload rows per core
BIG = 1e30


# ---------------------------------------------------------------------------
# host preprocessing
# ---------------------------------------------------------------------------

def _pack(x, edge_src, edge_dst, edge_weight):
    src = np.asarray(edge_src, np.int64)
    dst = np.asarray(edge_dst, np.int64)
    w = np.asarray(edge_weight, np.float32)
    x_bf = np.ascontiguousarray(np.asarray(x, np.float32)).astype(BFNP)

    # pass 1: per-core greedy bin boundaries + node->table-row map
    binrow = np.zeros((NCORES, NPC), np.int64)   # local node -> row in [0,NPAD)
    starts_all, counts_all = [], []
    for c in range(NCORES):
        lo = c * NPC
        m = (dst >= lo) & (dst < lo + NPC)
        ed = dst[m] - lo
        deg = np.bincount(ed, minlength=NPC)
        bstart, bnodes = [], []
        n0 = 0
        while n0 < NPC:
            e_acc, nn = 0, 0
            while n0 + nn < NPC and nn < 128 and e_acc + deg[n0 + nn] <= 2048:
                e_acc += deg[n0 + nn]
                nn += 1
            assert nn > 0
            bstart.append(n0)
            bnodes.append(nn)
            n0 += nn
        assert len(bstart) <= NCH, f"core {c} needs {len(bstart)} bins"
        while len(bstart) < NCH:
            bstart.append(NPC)
            bnodes.append(0)
        bstart = np.asarray(bstart, np.int64)
        bnodes = np.asarray(bnodes, np.int64)
        for b in range(NCH):
            s, nn = bstart[b], bnodes[b]
            binrow[c, s:s + nn] = b * 128 + np.arange(nn)
        starts_all.append(bstart)
        counts_all.append(bnodes)

    per_core = []
    for c in range(NCORES):
        lo = c * NPC
        m = (dst >= lo) & (dst < lo + NPC)
        es, ed, ew = src[m], dst[m] - lo, w[m]
        order = np.argsort(ed, kind="stable")
        es, ed, ew = es[order], ed[order], ew[order]
        bstart, bnodes = starts_all[c], counts_all[c]
        # edge ranges per bin (bins are consecutive node ranges)
        bin_edge_start = np.searchsorted(ed, bstart)
        bin_edge_end = np.searchsorted(ed, bstart + bnodes)

        # slot assignment (column-major within each bin's 2 blocks)
        srcslot = np.full(BTOT * 1024, -1, np.int64)
        dslot = np.zeros(BTOT * 1024, np.int64)
        wslot = np.zeros(BTOT * 1024, np.float32)
        for b in range(NCH):
            e0, e1 = bin_edge_start[b], bin_edge_end[b]
            cnt = e1 - e0
            assert cnt <= NB * 1024
            u = np.arange(cnt)
            blk = b * NB + u // 1024
            u2 = u % 1024
            pos = blk * 1024 + u2
            srcslot[pos] = es[e0:e1]
            dslot[pos] = ed[e0:e1] - bstart[b]
            wslot[pos] = ew[e0:e1]

        real = srcslot >= 0
        # xe: [128, BTOT*8*256] pre-gathered source rows, bf16
        rows = np.zeros((BTOT * 1024, FIN), BFNP)
        rows[real] = x_bf[srcslot[real]]
        xe = np.ascontiguousarray(
            rows.reshape(BTOT, 8, 128, FIN).transpose(2, 0, 1, 3)
            .reshape(128, BTOT * 8 * FIN))

        # woh: [128, BTOT*8*128] edge-weighted one-hots, bf16
        woh = np.zeros((128, BTOT * 8 * 128), np.float32)
        pos = np.nonzero(real)[0]
        blk = pos // 1024
        u2 = pos % 1024
        p = u2 % 128
        j = u2 // 128
        woh[p, (blk * 8 + j) * 128 + dslot[pos]] = wslot[pos]
        woh = woh.astype(BFNP)

        # idx2: slot -> row in the AllGather'd table
        sc = srcslot // NPC
        slo = srcslot - sc * NPC
        idx2 = np.zeros(BTOT * 1024, np.int64)
        idx2[real] = sc[real] * NPAD + binrow[sc[real], slo[real]]
        per_core.append(dict(xe=xe, woh=woh, idx2=idx2,
                             bstart=bstart, bnodes=bnodes))
    return per_core


def _wrap16(idx_flat):
    """[BTOT*1024] -> [128, BTOT*64] int16, per-block wrapped-16 replicated."""
    out = np.zeros((128, BTOT * 64), np.int16)
    for blk in range(BTOT):
        b = idx_flat[blk * 1024:(blk + 1) * 1024].astype(np.int16)
        t = b.reshape(64, 16).T          # [16, 64]
        out[:, blk * 64:(blk + 1) * 64] = np.tile(t, (8, 1))
    return out


def _host_inputs(inputs, prep):
    x = np.ascontiguousarray(np.asarray(inputs["x"], np.float32))

    def padT(a, rows, cols):
        out = np.zeros((rows, cols), np.float32)
        t = np.asarray(a, np.float32).T
        out[: t.shape[0], : t.shape[1]] = t
        return out.astype(BFNP)

    w1relT = padT(inputs["W1_rel"], FIN, HPAD)
    w1rootT = padT(inputs["W1_root"], FIN, HPAD)
    w2relT = padT(inputs["W2_rel"], HPAD, HPAD)
    w2rootT = padT(inputs["W2_root"], HPAD, HPAD)

    def rowv(v):
        out = np.zeros((1, HPAD), np.float32)
        vv = np.asarray(v, np.float32)
        out[0, : vv.shape[0]] = vv
        return out

    b1row = rowv(inputs["b1"]).astype(BFNP)
    b2row = rowv(inputs["b2"]).astype(BFNP)
    p1f = rowv(inputs["p1_w"])
    p2f = rowv(inputs["p2_w"])
    p1r = np.tile(p1f, (128, 1)).astype(BFNP)
    p2r = np.tile(p2f, (128, 1)).astype(BFNP)

    iotaB = np.tile(np.arange(NBINS, dtype=np.float32)[None, :], (128, 1))
    identb = np.eye(128, dtype=np.float32).astype(BFNP)
    identf = np.eye(128, dtype=np.float32)
    ones128f = np.ones((1, 128), np.float32)
    ones1b = np.ones((1, 128), np.float32).astype(BFNP)
    onesPb = np.ones((128, 1), np.float32).astype(BFNP)
    ones11 = np.ones((1, 1), np.float32).astype(BFNP)

    # mask of z positions inside the flattened z2ro AllGather payload
    g = np.arange(128 * (NCORES * ZRO // 128), dtype=np.int64)
    romask = ((g % ZRO) < NPAD).astype(np.float32).reshape(
        128, NCORES * ZRO // 128)

    lin1W = np.asarray(inputs["lin1_W"], np.float32)   # [2000, 1000]
    lin2W = np.asarray(inputs["lin2_W"], np.float32)   # [4000, 2000]
    lin3W = np.asarray(inputs["lin3_W"], np.float32)   # [100, 4000]
    lin1b = np.asarray(inputs["lin1_b"], np.float32)
    lin2b = np.asarray(inputs["lin2_b"], np.float32)
    lin3b = np.asarray(inputs["lin3_b"], np.float32)

    # lin1 replicated: rows = z layout [max 0:500 | pad | mean 512:1012 | pad]
    l1T = np.zeros((1024, 2048), np.float32)
    sh = lin1W.T                                       # [1000, 2000]
    l1T[:500, :2000] = sh[:500]
    l1T[512:1012, :2000] = sh[500:]
    l1T = l1T.astype(BFNP)
    b1h = np.zeros((128, 16), np.float32)
    b1h.T.flat[:2000] = lin1b

    per_core = []
    for c in range(NCORES):
        pr = prep[c]
        bstart, bnodes = pr["bstart"], pr["bnodes"]

        xT = np.zeros((FIN, NPAD), np.float32)
        padmask = np.zeros((128, NCH), np.float32)
        for b in range(NCH):
            s, nn = bstart[b], bnodes[b]
            if nn:
                xT[:, b * 128: b * 128 + nn] = x[c * NPC + s: c * NPC + s + nn].T
                padmask[:nn, b] = 1.0
        xT = xT.astype(BFNP)

        l2T = np.zeros((2048, 500), np.float32)
        l2T[:2000] = lin2W[c * 500:(c + 1) * 500].T
        l2T = l2T.astype(BFNP)
        b2h = np.zeros((128, 4), np.float32)
        b2h.T.flat[:500] = lin2b[c * 500:(c + 1) * 500]

        l3T = np.zeros((512, 128), np.float32)
        l3T[:500, :NOUT] = lin3W[:, c * 500:(c + 1) * 500].T
        l3T = l3T.astype(BFNP)
        b3h = np.zeros((128, 1), np.float32)
        b3h[:NOUT, 0] = lin3b

        per_core.append(dict(
            xe=pr["xe"], woh=pr["woh"], idx2=_wrap16(pr["idx2"]),
            padmask=padmask, xT=xT,
            w1relT=w1relT, w1rootT=w1rootT, w2relT=w2relT, w2rootT=w2rootT,
            b1row=b1row, b2row=b2row, p1f=p1f, p2f=p2f, p1r=p1r, p2r=p2r,
            iotaB=iotaB, identb=identb, identf=identf, ones128f=ones128f,
            ones1b=ones1b, onesPb=onesPb, ones11=ones11,
            romask=romask,
            l1T=l1T, b1h=b1h, l2T=l2T, b2h=b2h, l3T=l3T, b3h=b3h,
        ))
    return per_core


# ---------------------------------------------------------------------------
# device program
# ---------------------------------------------------------------------------

def _build():
    nc = bacc.Bacc("TRN2", target_bir_lowering=False, debug=False,
                   num_devices=NCORES)

    def din(name, shape, dt=FP32):
        return nc.dram_tensor(name, shape, dt, kind="ExternalInput")

    xe = din("xe", [128, BTOT * 8 * FIN], BF16)
    woh = din("woh", [128, BTOT * 8 * 128], BF16)
    idx2 = din("idx2", [128, BTOT * 64], I16)
    padmask = din("padmask", [128, NCH])
    xT = din("xT", [FIN, NPAD], BF16)
    w1relT = din("w1relT", [FIN, HPAD], BF16)
    w1rootT = din("w1rootT", [FIN, HPAD], BF16)
    w2relT = din("w2relT", [HPAD, HPAD], BF16)
    w2rootT = din("w2rootT", [HPAD, HPAD], BF16)
    b1row = din("b1row", [1, HPAD], BF16)
    b2row = din("b2row", [1, HPAD], BF16)
    p1f = din("p1f", [1, HPAD])
    p2f = din("p2f", [1, HPAD])
    p1r = din("p1r", [128, HPAD], BF16)
    p2r = din("p2r", [128, HPAD], BF16)
    iotaB = din("iotaB", [128, NBINS])
    identb = din("identb", [128, 128], BF16)
    identf = din("identf", [128, 128])
    ones128f = din("ones128f", [1, 128])
    ones1b = din("ones1b", [1, 128], BF16)
    onesPb = din("onesPb", [128, 1], BF16)
    ones11 = din("ones11", [1, 1], BF16)
    romask = din("romask", [128, NCORES * ZRO // 128])
    l1T = din("l1T", [1024, 2048], BF16)
    b1h = din("b1h", [128, 16])
    l2T = din("l2T", [2048, 500], BF16)
    b2h = din("b2h", [128, 4])
    l3T = din("l3T", [512, 128], BF16)
    b3h = din("b3h", [128, 1])

    out = nc.dram_tensor("out", [1, NOUT], FP32, kind="ExternalOutput")

    RG = [list(range(NCORES))]

    with tile.TileContext(nc) as tc:
        with (
            tc.tile_pool(name="const", bufs=1) as cp,
            tc.tile_pool(name="stream", bufs=3) as sp,
            tc.tile_pool(name="gather", bufs=2) as gp,
            tc.tile_pool(name="work", bufs=1) as wp,
            tc.tile_pool(name="big", bufs=1) as bigp,
            tc.tile_pool(name="psA", bufs=2, space="PSUM") as psA,
            tc.tile_pool(name="psB", bufs=2, space="PSUM") as psB,
            tc.tile_pool(name="psT", bufs=2, space="PSUM") as psT,
            tc.tile_pool(name="psS", bufs=1, space="PSUM") as psS,
            tc.tile_pool(name="dram", bufs=1, space="DRAM") as dr,
        ):
            def load(src, dt=FP32, tag=None):
                tl = cp.tile(list(src.shape), dt, tag=tag or src.name)
                nc.sync.dma_start(tl[:], src[:])
                return tl

            idx2_t = load(idx2, I16)
            pad_t = load(padmask)
            iob_t = load(iotaB)
            idb_t = load(identb, BF16)
            idf_t = load(identf)
            ones_t = load(ones128f)
            ones1b_t = load(ones1b, BF16)
            onesPb_t = load(onesPb, BF16)
            ones11_t = load(ones11, BF16)
            b1row_t = load(b1row, BF16)
            b2row_t = load(b2row, BF16)
            p1f_t = load(p1f)
            p2f_t = load(p2f)
            p1r_t = load(p1r, BF16)
            p2r_t = load(p2r, BF16)
            rom_t = load(romask)

            def load_chunks(src, nchunks, cols, tag, dt=BF16):
                ts = []
                for k in range(nchunks):
                    t = cp.tile([128, cols], dt, tag=f"{tag}{k}")
                    nc.sync.dma_start(t[:], src[k * 128:(k + 1) * 128, :cols])
                    ts.append(t)
                return ts

            w1rel_t = load_chunks(w1relT, 2, HPAD, "w1rel")
            w1root_t = load_chunks(w1rootT, 2, HPAD, "w1root")
            w2rel_t = load_chunks(w2relT, 4, HPAD, "w2rel")
            w2root_t = load_chunks(w2rootT, 4, HPAD, "w2root")
            xT_t = load_chunks(xT, 2, NPAD, "xTc")

            # DRAM internal tiles
            tbl = dr.tile([NPAD, ROWB], BF16)
            tblag = dr.tile([NROWS, ROWB], BF16, addr_space="Shared")
            zsh1 = dr.tile([NPAD, 1], FP32)
            zag1 = dr.tile([NROWS, 1], FP32, addr_space="Shared")
            z2ro = dr.tile([ZRO, 1], FP32)
            z2roag = dr.tile([NCORES * ZRO, 1], FP32, addr_space="Shared")
            ro2in = dr.tile([2, HPAD], FP32)
            ro2ag = dr.tile([2 * NCORES, HPAD], FP32, addr_space="Shared")
            oin = dr.tile([128, 1], FP32)
            oar = dr.tile([128, 1], FP32, addr_space="Shared")
            wrm = dr.tile([16, 1], FP32)
            wrmag = dr.tile([16 * NCORES, 1], FP32, addr_space="Shared")

            # collective-stack warmup: absorb first-collective setup cost
            # while layer 1 computes
            wz = wp.tile([16, 1], FP32, tag="wz")
            nc.vector.memset(wz[:], 0.0)
            nc.sync.dma_start(wrm[:], wz[:])
            nc.gpsimd.collective_compute(
                "AllGather", OP.bypass, replica_groups=RG,
                ins=[wrm[:]], outs=[wrmag[:]])

            # -------- norms first (Sqrt table load hides under L1) ---------
            def inv_norm_b(pf_t, lname):
                """[128,1] broadcast of 1/||p||."""
                sq = wp.tile([1, HPAD], FP32, tag="pnsq")
                nc.vector.tensor_tensor(out=sq[:], in0=pf_t[:], in1=pf_t[:],
                                        op=OP.mult)
                n2 = wp.tile([1, 1], FP32, tag="pn2")
                nc.vector.tensor_reduce(out=n2[:], in_=sq[:], op=OP.add,
                                        axis=AX.X)
                nc.scalar.activation(n2[:], n2[:], ACT.Sqrt)
                nc.vector.reciprocal(n2[:], n2[:])
                ib_ps = psS.tile([128, 1], FP32, tag="small")
                nc.tensor.matmul(out=ib_ps[:], lhsT=ones_t[:], rhs=n2[:],
                                 start=True, stop=True)
                ib = wp.tile([128, 1], FP32, tag=f"invbs{lname}")
                nc.vector.tensor_copy(ib[:], ib_ps[:])
                return ib

            inv1b = inv_norm_b(p1f_t, "l1")
            inv2b = inv_norm_b(p2f_t, "l2")

            # ---------------- histogram k-th threshold ---------------------
            NF2 = NCORES * ZRO // 128
            S_big = wp.tile([128, NF2 * NBINS], BF16, tag="Sbig")

            def topk_tau(zt, nfree, k, lname):
                """zt: [128, nfree] fp32 scores (pads/masked = -BIG).
                returns [128,1] tile with the k-th-largest threshold."""
                mm = wp.tile([128, 2], FP32, tag="mm")
                msk = wp.tile([128, nfree], FP32, tag=f"hmsk{lname}")
                nc.vector.tensor_scalar(msk[:], zt[:], -1e29, 2e30, OP.is_lt,
                                        OP.mult)
                nc.vector.tensor_tensor(out=msk[:], in0=msk[:], in1=zt[:],
                                        op=OP.add)
                nc.vector.tensor_reduce(out=mm[:, 0:1], in_=msk[:], op=OP.min,
                                        axis=AX.X)
                nc.vector.tensor_reduce(out=mm[:, 1:2], in_=zt[:], op=OP.max,
                                        axis=AX.X)
                ztb = wp.tile([128, nfree], BF16, tag=f"ztb{lname}")
                nc.vector.tensor_copy(ztb[:], zt[:])
                lw = wp.tile([1, 2], FP32, tag="lw")  # [lo, w]
                mmT = wp.tile([1, 2, 128], FP32, tag="mmTs")
                for col in range(2):
                    mmT_ps = psS.tile([1, 128], FP32, tag="small")
                    nc.tensor.transpose(out=mmT_ps[:], in_=mm[:, col:col + 1],
                                        identity=idf_t[:])
                    nc.vector.tensor_copy(mmT[:, col, :], mmT_ps[:])
                nc.vector.tensor_reduce(out=lw[:, 0:1], in_=mmT[:, 0, :],
                                        op=OP.min, axis=AX.X)
                nc.vector.tensor_reduce(out=lw[:, 1:2], in_=mmT[:, 1, :],
                                        op=OP.max, axis=AX.X)
                nc.vector.tensor_scalar_add(lw[:, 0:1], lw[:, 0:1], -1e-3)
                nc.vector.tensor_scalar_add(lw[:, 1:2], lw[:, 1:2], 1e-3)
                nc.vector.tensor_tensor(out=lw[:, 1:2], in0=lw[:, 1:2],
                                        in1=lw[:, 0:1], op=OP.subtract)
                nc.vector.tensor_scalar_mul(lw[:, 1:2], lw[:, 1:2], 1.0 / NBINS)

                for st in range(NSTAGES):
                    lwb_ps = psS.tile([128, 2], FP32, tag="small")
                    nc.tensor.matmul(out=lwb_ps[:], lhsT=ones_t[:], rhs=lw[:],
                                     start=True, stop=True)
                    lwb = wp.tile([128, 2], FP32, tag="lwbs")
                    nc.vector.tensor_copy(lwb[:], lwb_ps[:])
                    tt = wp.tile([128, NBINS], FP32, tag="tt")
                    nc.vector.tensor_scalar(tt[:], iob_t[:], lwb[:, 1:2],
                                            lwb[:, 0:1], OP.mult, OP.add)
                    ttb = wp.tile([128, NBINS], BF16, tag="ttb")
                    nc.vector.tensor_copy(ttb[:], tt[:])
                    # S[p, j, n]: count-reduce over n is contiguous
                    S = S_big[:, :NBINS * nfree].rearrange(
                        "p (j n) -> p j n", j=NBINS)
                    nc.vector.tensor_tensor(
                        out=S,
                        in0=ztb[:].unsqueeze(1).broadcast_to(
                            [128, NBINS, nfree]),
                        in1=ttb[:].unsqueeze(2).broadcast_to(
                            [128, NBINS, nfree]),
                        op=OP.is_ge)
                    cntp = wp.tile([128, NBINS], BF16, tag="cntp")
                    with nc.allow_low_precision(
                            reason="counts <= nfree are exact in bf16"):
                        nc.vector.tensor_reduce(
                            out=cntp[:], in_=S, op=OP.add, axis=AX.X)
                    cnt_ps = psS.tile([1, NBINS], FP32, tag="small")
                    nc.tensor.matmul(out=cnt_ps[:], lhsT=onesPb_t[:],
                                     rhs=cntp[:], start=True, stop=True)
                    fl = wp.tile([1, NBINS], FP32, tag="fl")
                    nc.vector.tensor_scalar(fl[:], cnt_ps[:], float(k), None,
                                            OP.is_ge)
                    js = wp.tile([1, 1], FP32, tag="js")
                    nc.vector.tensor_reduce(out=js[:], in_=fl[:], op=OP.add,
                                            axis=AX.X)
                    nc.vector.tensor_scalar_add(js[:], js[:], -1.0)
                    nc.vector.tensor_scalar(lw[:, 0:1], js[:], lw[:, 1:2],
                                            lw[:, 0:1], OP.mult, OP.add)
                    if st != NSTAGES - 1:
                        nc.vector.tensor_scalar_mul(lw[:, 1:2], lw[:, 1:2],
                                                    1.0 / NBINS)
                taub_ps = psS.tile([128, 1], FP32, tag="small")
                nc.tensor.matmul(out=taub_ps[:], lhsT=ones_t[:],
                                 rhs=lw[:, 0:1], start=True, stop=True)
                taub = wp.tile([128, 1], FP32, tag=f"taubs{lname}")
                nc.vector.tensor_copy(taub[:], taub_ps[:])
                return taub

            # ======================= layer 1 ===============================
            h1 = bigp.tile([128, NCH * HPAD], BF16, tag="h1_all")
            z1 = wp.tile([128, NCH], FP32, tag="z1")
            aggT1 = [bigp.tile([128, NPAD], BF16, tag=f"aggT1_{fc}",
                               name=f"aggT1_{fc}")
                     for fc in range(2)]

            for b in range(NCH):
                agg_ps = psA.tile([128, HPAD], FP32, tag="aggps")
                for k in range(NB):
                    B = b * NB + k
                    xeb = sp.tile([128, 8, FIN], BF16, tag="xeb")
                    nc.sync.dma_start(
                        xeb[:].rearrange("p a f -> p (a f)"),
                        xe[:, B * 8 * FIN:(B + 1) * 8 * FIN])
                    wohb = sp.tile([128, 8, 128], BF16, tag="wohb1")
                    nc.sync.dma_start(
                        wohb[:].rearrange("p a d -> p (a d)"),
                        woh[:, B * 1024:(B + 1) * 1024])
                    for j in range(8):
                        nc.tensor.matmul(
                            out=agg_ps[:, :FIN], lhsT=wohb[:, j, :],
                            rhs=xeb[:, j, :],
                            start=(k == 0 and j == 0),
                            stop=(k == NB - 1 and j == 7))
                # transpose agg -> aggT1 chunks
                aggc = wp.tile([128, FIN], BF16, tag="aggc", bufs=2)
                nc.scalar.activation(aggc[:], agg_ps[:, :FIN], ACT.Copy)
                for fc in range(2):
                    tps = psT.tile([128, 128], BF16, tag="tps")
                    nc.tensor.transpose(out=tps[:],
                                        in_=aggc[:, fc * 128:(fc + 1) * 128],
                                        identity=idb_t[:])
                    nc.scalar.activation(aggT1[fc][:, b * 128:(b + 1) * 128],
                                         tps[:], ACT.Copy)
                # dense: h = relu(b1 + aggT.T @ w1relT + xT.T @ w1rootT)
                hp = psB.tile([128, HPAD], FP32, tag="hps")
                nc.tensor.matmul(out=hp[:], lhsT=ones1b_t[:], rhs=b1row_t[:],
                                 start=True, stop=False)
                for fc in range(2):
                    nc.tensor.matmul(
                        out=hp[:], lhsT=aggT1[fc][:, b * 128:(b + 1) * 128],
                        rhs=w1rel_t[fc][:], start=False, stop=False)
                for fc in range(2):
                    nc.tensor.matmul(
                        out=hp[:], lhsT=xT_t[fc][:, b * 128:(b + 1) * 128],
                        rhs=w1root_t[fc][:], start=False, stop=(fc == 1))
                hc = h1[:, b * HPAD:(b + 1) * HPAD]
                nc.scalar.activation(hc, hp[:], ACT.Relu)
                # z score (fp32)
                scr = wp.tile([128, HPAD], FP32, tag="scr", bufs=2)
                nc.vector.tensor_tensor(out=scr[:], in0=hc, in1=p1r_t[:],
                                        op=OP.mult)
                nc.vector.tensor_reduce(out=z1[:, b:b + 1], in_=scr[:],
                                        op=OP.add, axis=AX.X)
                # table row: [h | z | pad]
                tblb = wp.tile([128, ROWB], BF16, tag="tblb", bufs=2)
                nc.scalar.activation(tblb[:, 0:HPAD], hp[:], ACT.Relu)
                nc.vector.tensor_copy(
                    tblb[:, HPAD:HPAD + 2].bitcast(FP32), z1[:, b:b + 1])
                nc.sync.dma_start(tbl[b * 128:(b + 1) * 128, :], tblb[:])

            # masked z for selection
            pm30 = wp.tile([128, NCH], FP32, tag="pm30")
            nc.vector.tensor_scalar(pm30[:], pad_t[:], 1.0, BIG, OP.subtract,
                                    OP.mult)
            zm1 = wp.tile([128, NCH], FP32, tag="zm1")
            nc.vector.tensor_tensor(out=zm1[:], in0=z1[:], in1=pad_t[:],
                                    op=OP.mult)
            nc.vector.tensor_tensor(out=zm1[:], in0=zm1[:], in1=pm30[:],
                                    op=OP.add)
            nc.sync.dma_start(
                zsh1[:].rearrange("(b p) o -> p (b o)", p=128), zm1[:])
            nc.gpsimd.collective_compute(
                "AllGather", OP.bypass, replica_groups=RG,
                ins=[tbl[:]], outs=[tblag[:]])
            nc.gpsimd.collective_compute(
                "AllGather", OP.bypass, replica_groups=RG,
                ins=[zsh1[:]], outs=[zag1[:]])

            zt1 = wp.tile([128, NROWS // 128], FP32, tag="zt1")
            nc.sync.dma_start(
                zt1[:], zag1[:].rearrange("(p f) o -> p (f o)", p=128))
            tau1b = topk_tau(zt1, NROWS // 128, K1, "l1")

            # a1 per local bin + kept masks
            kp1 = wp.tile([128, NCH], FP32, tag="kp1")
            nc.vector.tensor_scalar(kp1[:], zm1[:], tau1b[:, 0:1], None,
                                    OP.is_ge)
            s1 = wp.tile([128, NCH], FP32, tag="s1")
            nc.scalar.activation(s1[:], z1[:], ACT.Tanh, scale=inv1b[:, 0:1])
            a1 = wp.tile([128, NCH], FP32, tag="a1")
            nc.vector.tensor_tensor(out=a1[:], in0=s1[:], in1=kp1[:],
                                    op=OP.mult)
            km30 = wp.tile([128, NCH], FP32, tag="km30")
            nc.vector.tensor_scalar(km30[:], kp1[:], 1.0, BIG, OP.subtract,
                                    OP.mult)

            # g1 (scaled, masked transpose) + readout 1
            gmT1 = [bigp.tile([128, NPAD], BF16, tag=f"gmT1_{fc}",
                              name=f"gmT1_{fc}")
                    for fc in range(4)]
            ro1s_ps = psS.tile([1, HPAD], FP32, tag="rosum")
            for b in range(NCH):
                hc = h1[:, b * HPAD:(b + 1) * HPAD]
                g1c = wp.tile([128, HPAD], BF16, tag="g1c", bufs=2)
                nc.vector.tensor_scalar(g1c[:], hc, a1[:, b:b + 1], None,
                                        OP.mult)
                nc.tensor.matmul(out=ro1s_ps[:], lhsT=onesPb_t[:], rhs=g1c[:],
                                 start=(b == 0), stop=(b == NCH - 1))
                gmc = wp.tile([128, HPAD], BF16, tag="gmc", bufs=2)
                nc.vector.tensor_scalar(gmc[:], hc, a1[:, b:b + 1],
                                        km30[:, b:b + 1], OP.mult, OP.add)
                for fc in range(4):
                    tps = psT.tile([128, 128], BF16, tag="tps")
                    nc.tensor.transpose(out=tps[:],
                                        in_=gmc[:, fc * 128:(fc + 1) * 128],
                                        identity=idb_t[:])
                    nc.scalar.activation(gmT1[fc][:, b * 128:(b + 1) * 128],
                                         tps[:], ACT.Copy)
            m1T = wp.tile([128, 4], FP32, tag="m1T")
            for fc in range(4):
                nc.vector.tensor_reduce(out=m1T[:, fc:fc + 1], in_=gmT1[fc][:],
                                        op=OP.max, axis=AX.X)
            ro1s = wp.tile([1, HPAD], FP32, tag="ro1s")
            nc.vector.tensor_copy(ro1s[:], ro1s_ps[:])
            # ro1 rides in the z2ro payload (rows NPAD.. and NPAD+512..)
            nc.sync.dma_start(z2ro[NPAD:NPAD + HPAD, :]
                              .rearrange("f o -> o f"), ro1s[:])
            nc.sync.dma_start(
                z2ro[NPAD + HPAD:NPAD + 1024, :]
                .rearrange("(c p) o -> p (c o)", p=128), m1T[:])

            # ======================= layer 2 ===============================
            h2 = bigp.tile([128, NCH * HPAD], BF16, tag="h2_all")
            z2 = wp.tile([128, NCH], FP32, tag="z2")
            aggT2 = [bigp.tile([128, NPAD], BF16, tag=f"aggT2_{fc}",
                               name=f"aggT2_{fc}")
                     for fc in range(4)]

            for b in range(NCH):
                agg_ps = psA.tile([128, HPAD], FP32, tag="aggps")
                for k in range(NB):
                    B = b * NB + k
                    gt = gp.tile([128, 8, ROWB], BF16, tag="gath", bufs=3)
                    nc.gpsimd.dma_gather(
                        gt[:], tblag[:], idx2_t[:, B * 64:(B + 1) * 64],
                        1024, 1024, ROWB)
                    wohb = sp.tile([128, 8, 128], BF16, tag="wohb2")
                    nc.sync.dma_start(
                        wohb[:].rearrange("p a d -> p (a d)"),
                        woh[:, B * 1024:(B + 1) * 1024])
                    # per-slot scale a1 = tanh(z*inv)*(z>=tau)
                    zg = gt[:, :, HPAD:HPAD + 2].bitcast(FP32) \
                        .rearrange("p a o -> p (a o)")
                    kp8 = wp.tile([128, 8], FP32, tag="kp8", bufs=2)
                    nc.vector.tensor_scalar(kp8[:], zg, tau1b[:, 0:1], None,
                                            OP.is_ge)
                    s8 = wp.tile([128, 8], FP32, tag="s8", bufs=2)
                    nc.scalar.activation(s8[:], zg, ACT.Tanh,
                                         scale=inv1b[:, 0:1])
                    a1s = wp.tile([128, 8], BF16, tag="a1s", bufs=2)
                    nc.vector.tensor_tensor(out=a1s[:], in0=s8[:], in1=kp8[:],
                                            op=OP.mult)
                    ohs = wp.tile([128, 8, 128], BF16, tag="ohs", bufs=2)
                    nc.vector.tensor_tensor(
                        out=ohs[:], in0=wohb[:],
                        in1=a1s[:].unsqueeze(2).broadcast_to([128, 8, 128]),
                        op=OP.mult)
                    for j in range(8):
                        nc.tensor.matmul(
                            out=agg_ps[:], lhsT=ohs[:, j, :],
                            rhs=gt[:, j, 0:HPAD],
                            start=(k == 0 and j == 0),
                            stop=(k == NB - 1 and j == 7))
                aggc = wp.tile([128, HPAD], BF16, tag="aggc2", bufs=2)
                nc.scalar.activation(aggc[:], agg_ps[:], ACT.Copy)
                for fc in range(4):
                    tps = psT.tile([128, 128], BF16, tag="tps")
                    nc.tensor.transpose(out=tps[:],
                                        in_=aggc[:, fc * 128:(fc + 1) * 128],
                                        identity=idb_t[:])
                    nc.scalar.activation(aggT2[fc][:, b * 128:(b + 1) * 128],
                                         tps[:], ACT.Copy)
                hp = psB.tile([128, HPAD], FP32, tag="hps")
                nc.tensor.matmul(out=hp[:], lhsT=ones1b_t[:], rhs=b2row_t[:],
                                 start=True, stop=False)
                for fc in range(4):
                    nc.tensor.matmul(
                        out=hp[:], lhsT=aggT2[fc][:, b * 128:(b + 1) * 128],
                        rhs=w2rel_t[fc][:], start=False, stop=False)
                for fc in range(4):
                    nc.tensor.matmul(
                        out=hp[:], lhsT=gmT1[fc][:, b * 128:(b + 1) * 128],
                        rhs=w2root_t[fc][:], start=False, stop=(fc == 3))
                hc = h2[:, b * HPAD:(b + 1) * HPAD]
                nc.scalar.activation(hc, hp[:], ACT.Relu)
                scr = wp.tile([128, HPAD], FP32, tag="scr", bufs=2)
                nc.vector.tensor_tensor(out=scr[:], in0=hc, in1=p2r_t[:],
                                        op=OP.mult)
                nc.vector.tensor_reduce(out=z2[:, b:b + 1], in_=scr[:],
                                        op=OP.add, axis=AX.X)

            # masked z2 (kept-in-l1 only) -> z2ro payload -> AllGather
            zm2 = wp.tile([128, NCH], FP32, tag="zm2")
            nc.vector.tensor_tensor(out=zm2[:], in0=z2[:], in1=kp1[:],
                                    op=OP.mult)
            nc.vector.tensor_tensor(out=zm2[:], in0=zm2[:], in1=km30[:],
                                    op=OP.add)
            nc.sync.dma_start(
                z2ro[0:NPAD, :].rearrange("(b p) o -> p (b o)", p=128),
                zm2[:])
            nc.gpsimd.collective_compute(
                "AllGather", OP.bypass, replica_groups=RG,
                ins=[z2ro[:]], outs=[z2roag[:]])

            # tau2 over the masked flat payload
            ztr = wp.tile([128, NF2], FP32, tag="ztr")
            nc.sync.dma_start(
                ztr[:], z2roag[:].rearrange("(p f) o -> p (f o)", p=128))
            zt2 = wp.tile([128, NF2], FP32, tag="zt2")
            nc.vector.tensor_tensor(out=zt2[:], in0=ztr[:], in1=rom_t[:],
                                    op=OP.mult)
            rm30 = wp.tile([128, NF2], FP32, tag="rm30")
            nc.vector.tensor_scalar(rm30[:], rom_t[:], 1.0, BIG, OP.subtract,
                                    OP.mult)
            nc.vector.tensor_tensor(out=zt2[:], in0=zt2[:], in1=rm30[:],
                                    op=OP.add)
            tau2b = topk_tau(zt2, NF2, K2, "l2")

            kp2 = wp.tile([128, NCH], FP32, tag="kp2")
            nc.vector.tensor_scalar(kp2[:], zm2[:], tau2b[:, 0:1], None,
                                    OP.is_ge)
            s2 = wp.tile([128, NCH], FP32, tag="s2")
            nc.scalar.activation(s2[:], z2[:], ACT.Tanh, scale=inv2b[:, 0:1])
            a2 = wp.tile([128, NCH], FP32, tag="a2")
            nc.vector.tensor_tensor(out=a2[:], in0=s2[:], in1=kp2[:],
                                    op=OP.mult)
            km30b = wp.tile([128, NCH], FP32, tag="km30b")
            nc.vector.tensor_scalar(km30b[:], kp2[:], 1.0, BIG, OP.subtract,
                                    OP.mult)

            ro2s_ps = psS.tile([1, HPAD], FP32, tag="rosum")
            m2T = wp.tile([128, 4], FP32, tag="m2T")
            nc.vector.memset(m2T[:], -1e30)
            for b in range(NCH):
                hc = h2[:, b * HPAD:(b + 1) * HPAD]
                g2c = wp.tile([128, HPAD], BF16, tag="g1c", bufs=2)
                nc.vector.tensor_scalar(g2c[:], hc, a2[:, b:b + 1], None,
                                        OP.mult)
                nc.tensor.matmul(out=ro2s_ps[:], lhsT=onesPb_t[:], rhs=g2c[:],
                                 start=(b == 0), stop=(b == NCH - 1))
                gmc = wp.tile([128, HPAD], BF16, tag="gmc", bufs=2)
                nc.vector.tensor_scalar(gmc[:], hc, a2[:, b:b + 1],
                                        km30b[:, b:b + 1], OP.mult, OP.add)
                for fc in range(4):
                    tps = psT.tile([128, 128], BF16, tag="tps")
                    nc.tensor.transpose(out=tps[:],
                                        in_=gmc[:, fc * 128:(fc + 1) * 128],
                                        identity=idb_t[:])
                    red = wp.tile([128, 1], FP32, tag="redm", bufs=2)
                    nc.vector.tensor_reduce(out=red[:], in_=tps[:],
                                            op=OP.max, axis=AX.X)
                    nc.vector.tensor_tensor(out=m2T[:, fc:fc + 1],
                                            in0=m2T[:, fc:fc + 1],
                                            in1=red[:], op=OP.max)
            ro2s = wp.tile([1, HPAD], FP32, tag="ro2s")
            nc.vector.tensor_copy(ro2s[:], ro2s_ps[:])
            nc.sync.dma_start(ro2in[0:1, :], ro2s[:])
            nc.sync.dma_start(
                ro2in[1:2, :].rearrange("o (c p) -> p (o c)", p=128), m2T[:])
            nc.gpsimd.collective_compute(
                "AllGather", OP.bypass, replica_groups=RG,
                ins=[ro2in[:]], outs=[ro2ag[:]])

            # ======================= readout combine + head ================
            # ro1 lives in z2roag rows [s*ZRO+NPAD, s*ZRO+NPAD+1024)
            mx1 = wp.tile([128, 4], FP32, tag="mx1")
            mn1 = wp.tile([128, 4], FP32, tag="mn1")
            sums1 = wp.tile([128, 4, NCORES], FP32, tag="cmb1")
            maxs1 = wp.tile([128, 4, NCORES], FP32, tag="cmbm1")
            for s in range(NCORES):
                base = s * ZRO + NPAD
                nc.sync.dma_start(
                    sums1[:, :, s],
                    z2roag[base:base + HPAD, :]
                    .rearrange("(c p) o -> p (c o)", p=128))
                nc.sync.dma_start(
                    maxs1[:, :, s],
                    z2roag[base + HPAD:base + 1024, :]
                    .rearrange("(c p) o -> p (c o)", p=128))
            nc.vector.tensor_reduce(out=mn1[:], in_=sums1[:], op=OP.add,
                                    axis=AX.X)
            nc.vector.tensor_reduce(out=mx1[:], in_=maxs1[:], op=OP.max,
                                    axis=AX.X)
            nc.vector.tensor_scalar_mul(mn1[:], mn1[:], 1.0 / K1)

            mx2 = wp.tile([128, 4], FP32, tag="mx2")
            mn2 = wp.tile([128, 4], FP32, tag="mn2")
            sums2 = wp.tile([128, 4, 2 * NCORES], FP32, tag="cmb2")
            for r in range(2 * NCORES):
                nc.sync.dma_start(
                    sums2[:, :, r],
                    ro2ag[r:r + 1, :].rearrange("o (c p) -> p (o c)", p=128))
            s_ap = sums2[:].rearrange("p c (s t) -> p c t s", t=2)
            nc.vector.tensor_reduce(out=mn2[:], in_=s_ap[:, :, 0, :],
                                    op=OP.add, axis=AX.X)
            nc.vector.tensor_reduce(out=mx2[:], in_=s_ap[:, :, 1, :],
                                    op=OP.max, axis=AX.X)
            nc.vector.tensor_scalar_mul(mn2[:], mn2[:], 1.0 / K2)

            zT = wp.tile([128, 8], FP32, tag="zT")
            nc.vector.tensor_tensor(out=zT[:, 0:4], in0=mx1[:], in1=mx2[:],
                                    op=OP.add)
            nc.vector.tensor_tensor(out=zT[:, 4:8], in0=mn1[:], in1=mn2[:],
                                    op=OP.add)
            zTb = wp.tile([128, 8], BF16, tag="zTb")
            nc.vector.tensor_copy(zTb[:], zT[:])

            # lin1 replicated: z1hT [128, 16] (matvec, outputs on partitions)
            b1h_t = load(b1h)
            z1hT = wp.tile([128, 16], BF16, tag="z1hT")
            o1p = psB.tile([128, 16], FP32, tag="hps")
            for t in range(8):
                l1c = sp.tile([128, 2048], BF16, tag="l1s")
                nc.sync.dma_start(l1c[:], l1T[t * 128:(t + 1) * 128, :])
                for m in range(16):
                    nc.tensor.matmul(
                        out=o1p[:, m:m + 1],
                        lhsT=l1c[:, m * 128:(m + 1) * 128],
                        rhs=zTb[:, t:t + 1],
                        start=(t == 0 and m == 0), stop=(t == 7 and m == 15))
            for m in range(16):
                nc.scalar.activation(z1hT[:, m:m + 1], o1p[:, m:m + 1],
                                     ACT.Relu, bias=b1h_t[:, m:m + 1])

            # lin2 shard: z2hT [128, 4] (500 rows via msl slices)
            b2h_t = load(b2h)
            z2hT = wp.tile([128, 4], BF16, tag="z2hT")
            nc.vector.memset(z2hT[:], 0.0)
            msl = [(0, 128), (128, 256), (256, 384), (384, 500)]
            o2p = psB.tile([128, 4], FP32, tag="hps")
            for t in range(16):
                l2c = sp.tile([128, 500], BF16, tag="l2s")
                nc.sync.dma_start(l2c[:], l2T[t * 128:(t + 1) * 128, :])
                for mi, (m0, m1) in enumerate(msl):
                    nc.tensor.matmul(out=o2p[: m1 - m0, mi:mi + 1],
                                     lhsT=l2c[:, m0:m1],
                                     rhs=z1hT[:, t:t + 1],
                                     start=(t == 0 and mi == 0),
                                     stop=(t == 15 and mi == 3))
            for mi, (m0, m1) in enumerate(msl):
                nc.scalar.activation(z2hT[: m1 - m0, mi:mi + 1],
                                     o2p[: m1 - m0, mi:mi + 1],
                                     ACT.Relu, bias=b2h_t[: m1 - m0, mi:mi + 1])

            # lin3 partial (own contraction shard) + AllReduce
            l3_t = load_chunks(l3T, 4, 128, "l3Tc")
            b3h_t = load(b3h)
            o3p = psB.tile([128, 1], FP32, tag="hps")
            for t in range(4):
                nc.tensor.matmul(out=o3p[:], lhsT=l3_t[t][:],
                                 rhs=z2hT[:, t:t + 1],
                                 start=(t == 0), stop=(t == 3))
            o3 = wp.tile([128, 1], FP32, tag="o3")
            nc.vector.tensor_copy(o3[:], o3p[:])
            nc.sync.dma_start(oin[:], o3[:])
            nc.gpsimd.collective_compute(
                "AllReduce", OP.add, replica_groups=RG,
                ins=[oin[:]], outs=[oar[:]])
            fin = wp.tile([128, 1], FP32, tag="fin")
            nc.sync.dma_start(fin[:], oar[:])
            nc.scalar.activation(fin[:], fin[:], ACT.Sigmoid,
                                 bias=b3h_t[:, 0:1])
            nc.sync.dma_start(out[:].rearrange("o f -> f o"), fin[:NOUT, :])

    nc.compile()
    return nc


# ---------------------------------------------------------------------------
# entry point
# ---------------------------------------------------------------------------

_CACHE = {}
TRACE = False


def kernel(**inputs):
    prep = _pack(inputs["x"], inputs["edge_src"], inputs["edge_dst"],
                 inputs["edge_weight"])
    if "nc" not in _CACHE:
        _CACHE["nc"] = _build()
    nc = _CACHE["nc"]
    in_maps = _host_inputs(inputs, prep)
    res = bass_utils.run_bass_kernel_spmd(
        nc, in_maps, core_ids=list(range(NCORES)), trace=TRACE)
    kernel.last_results = res
    return res.results[0]["out"]


if __name__ == "__main__":
    dat = np.load("/tmp/inputs.npz")
    inputs = {k: dat[k] for k in dat.files}
    got = kernel(**inputs)
    exp = np.load("/tmp/expected.npy")
    err = np.abs(got - exp).max()
    rel = err / np.abs(exp).max()
    print("out[0,:6] =", got[0, :6])
    print("exp[0,:6] =", exp[0, :6])
    print("max abs err:", err, "rel:", rel)
